# revision 30
# baseline (speedup 1.0000x reference)
"""MLA-style latent attention kernel for Trainium2, 8-core SPMD.

Problem: B=4, S=2048, H=2048, NH=16, HD=64, KVC=512, causal softmax attention.

Sharding: core i handles batch b = i//2 and head-half hp = i%2 (8 heads).
Each core computes its partial c_proj output (contraction over its 512 of the
1024 attn-out dims); the host sums the two partials per batch.

Per-core dataflow (all on one NeuronCore, fp32 I/O, fp32r matmuls):
  hidden [S,H] --PE transpose--> hiddenT [H,S] (streamed in s-chunks)
  qT   [512,S] = wq_l^T   @ hiddenT   (lhsT=wq_l,  rhs=hiddenT)
  latT [512,S] = wkv^T    @ hiddenT
  kT   [512,S] = wk_l^T   @ latT
  v    [S,512] (natural; stored as v1 [S, 8*(64+1)] with a ones column per
               head so the AV matmul also produces the softmax denominator)
  per head h, per 512-query block:
    S^T tile [128j, 512i] = kT_h-block^T(stationary) x qT_h-block(moving)
    P^T = exp(0.125*S^T) * causal-mask      (ACT exp, DVE mask-mul)
    [O^T; denom] += [v_h | 1]^T x P^T       (PSUM accumulate over j-tiles)
    OT_h = O^T * (1/denom)                  (DVE reciprocal + bcast mul)
  out [S,H] partial = OT^T(stationary per s-tile) x wo_l(moving)
"""

import os
import sys

import numpy as np

for _p in ("/opt/trn_rl_repo",):
    if os.path.isdir(_p) and _p not in sys.path:
        sys.path.append(_p)

import concourse.bass as bass  # noqa: E402
import concourse.mybir as mybir  # noqa: E402
from concourse import bacc, tile  # noqa: E402
from concourse.bass_utils import run_bass_kernel_spmd  # noqa: E402
from concourse.masks import make_identity  # noqa: E402

F32 = mybir.dt.float32
F32R = mybir.dt.float32r
BF16 = mybir.dt.bfloat16

B, S, H = 4, 2048, 2048
NH, HD = 16, 64
KVC = 512
DL = 512          # local q/k/v dims per core (8 heads x 64)
NHL = 8           # local heads
P = 128
SCALE = 0.125

_CACHE = {}


def build_program():
    """Build + compile the per-core Bass program. Returns the Bacc module."""
    nc = bacc.Bacc("TRN2", target_bir_lowering=False, debug=False)

    hs = nc.dram_tensor("hs", [S, H], F32, kind="ExternalInput").ap()
    wq = nc.dram_tensor("wq", [H, DL], F32R, kind="ExternalInput").ap()
    wkv = nc.dram_tensor("wkv", [H, KVC], F32R, kind="ExternalInput").ap()
    wk = nc.dram_tensor("wk", [KVC, DL], F32R, kind="ExternalInput").ap()
    wv = nc.dram_tensor("wv", [KVC, DL], F32R, kind="ExternalInput").ap()
    wo = nc.dram_tensor("wo", [DL, H], F32R, kind="ExternalInput").ap()
    out = nc.dram_tensor("out", [S, H], F32, kind="ExternalOutput").ap()

    HT = H // P      # 16 h-tiles
    DT = DL // P     # 4 d-tiles
    CT = KVC // P    # 4 c-tiles
    ST = S // P      # 16 s-tiles
    SC = 256         # pass-1 s-chunk
    NCH = S // SC    # 8 chunks

    from contextlib import ExitStack

    with tile.TileContext(nc) as tc, ExitStack() as stack:
        consts = stack.enter_context(tc.tile_pool(name="consts", bufs=1))
        ident = consts.tile([P, P], F32)
        make_identity(nc, ident)
        # bigmask[j, z] = 1.0 if z - j >= 384 else 0.  The causal mask for a
        # diagonal-band tile with j0 - i0 = t is bigmask[:, 384-t : 896-t].
        bigmask = consts.tile([P, 896], BF16)
        nc.gpsimd.memset(bigmask, 1.0)
        nc.gpsimd.affine_select(
            out=bigmask, in_=bigmask,
            compare_op=mybir.AluOpType.is_ge, fill=0.0,
            base=-384, pattern=[[1, 896]], channel_multiplier=-1,
        )

        persistA = stack.enter_context(tc.tile_pool(name="persistA", bufs=1))
        qT = persistA.tile([P, DT, S], F32R, tag="qT")
        latT = persistA.tile([P, DT, S], F32R, tag="latT")

        # ---------------- phase A: hiddenT -> qT, latT ----------------
        with tc.tile_pool(name="wA", bufs=1) as wA, \
             tc.tile_pool(name="hin", bufs=2) as hinp, \
             tc.tile_pool(name="hTp", bufs=2) as hTp, \
             tc.tile_pool(name="psA_t", bufs=2, space="PSUM") as psA_t, \
             tc.tile_pool(name="psA_m", bufs=2, space="PSUM") as psA_m:
            # weights go on the gpsimd (SWDGE) queue so the first hidden
            # chunk isn't stuck behind 8MB of weight DMA on the sync queue
            wq_sb = wA.tile([P, HT, DL], F32R, tag="wq")
            nc.gpsimd.dma_start(wq_sb, wq.rearrange("(a p) d -> p a d", p=P))
            wkv_sb = wA.tile([P, HT, KVC], F32R, tag="wkv")
            nc.gpsimd.dma_start(wkv_sb, wkv.rearrange("(a p) d -> p a d", p=P))

            for c in range(NCH):
                hin = hinp.tile([P, SC // P, H], F32, tag="hin")
                nc.sync.dma_start(
                    hin, hs[c * SC:(c + 1) * SC, :].rearrange(
                        "(sp p) h -> p sp h", p=P))
                hT = hTp.tile([P, HT, SC], F32R, tag="hT")
                for sp in range(SC // P):
                    for ht4 in range(HT // 4):
                        # pack 4 transposes into one PSUM bank, evict with a
                        # single strided DVE copy
                        ptr = psA_t.tile([P, 4, P], F32, tag="ptr")
                        for k in range(4):
                            ht = ht4 * 4 + k
                            nc.tensor.transpose(
                                ptr[:, k, :], hin[:, sp, ht * P:(ht + 1) * P],
                                ident)
                        nc.vector.tensor_copy(
                            hT[:, ht4 * 4:ht4 * 4 + 4, sp * P:(sp + 1) * P],
                            ptr)
                for dt_ in range(DT):
                    ps = psA_m.tile([P, SC], F32, tag="psq")
                    for ht in range(HT):
                        nc.tensor.matmul(
                            ps, wq_sb[:, ht, dt_ * P:(dt_ + 1) * P],
                            hT[:, ht, :],
                            start=(ht == 0), stop=(ht == HT - 1))
                    nc.vector.tensor_copy(qT[:, dt_, c * SC:(c + 1) * SC], ps)
                    ps2 = psA_m.tile([P, SC], F32, tag="pslat")
                    for ht in range(HT):
                        nc.tensor.matmul(
                            ps2, wkv_sb[:, ht, dt_ * P:(dt_ + 1) * P],
                            hT[:, ht, :],
                            start=(ht == 0), stop=(ht == HT - 1))
                    nc.vector.tensor_copy(
                        latT[:, dt_, c * SC:(c + 1) * SC], ps2)

        # ------- fused phase: A2 (kT/v1) + B (attention) + C (c_proj) -------
        # Wavefront over ib: the A2 chunk sc=ib produces exactly the kT/v1
        # s-range that B's query block ib needs, and C drains the OT block
        # that B(ib) finished, so PE work (A2/C) fills the ACT-bound gaps of
        # B.  All phase-B matmuls use PE tile mode (64,128): scores have a
        # 64-deep contraction and the AV matmuls are split into two 64-deep
        # halves; a uniform mode avoids per-matmul PE reconfiguration drains,
        # and the even/odd head operands live on SBUF partitions 0-63/64-127,
        # so their matmuls run concurrently on the T0/T8 array halves.
        persistB = stack.enter_context(tc.tile_pool(name="persistB", bufs=1))
        kT = persistB.tile([P, DT, S], F32R, tag="kT")
        v1 = persistB.tile([P, ST, NHL * (HD + 1)], BF16, tag="v1")
        OT = latT  # latT[ib-block] is dead once A2(ib) ran; reuse as OT

        with tc.tile_pool(name="wB", bufs=1) as wB, \
             tc.tile_pool(name="wo_p", bufs=1) as wop, \
             tc.tile_pool(name="ptp", bufs=3) as ptp, \
             tc.tile_pool(name="smallp", bufs=2) as smallp, \
             tc.tile_pool(name="osb_p", bufs=2) as osbp, \
             tc.tile_pool(name="ps_s", bufs=2, space="PSUM") as ps_sp, \
             tc.tile_pool(name="ps_o", bufs=1, space="PSUM") as ps_op:
            wk_sb = wB.tile([P, CT, DL], F32R, tag="wk")
            nc.gpsimd.dma_start(wk_sb, wk.rearrange("(a p) d -> p a d", p=P))
            wv_sb = wB.tile([P, CT, DL], F32R, tag="wv")
            nc.gpsimd.dma_start(wv_sb, wv.rearrange("(a p) d -> p a d", p=P))
            wo_sb = wop.tile([P, DT, H], F32R, tag="wo")
            nc.gpsimd.dma_start(wo_sb, wo.rearrange("(a p) n -> p a n", p=P))
            # ones columns (column HD of each head's 65-wide slot); DVE
            # memset can't write bf16-from-imm reliably, stage in f32
            ones_src = wB.tile([P, ST, NHL], F32, tag="ones_src")
            nc.vector.memset(ones_src, 1.0)
            nc.vector.tensor_copy(
                v1.rearrange("p s (h e) -> p s h e", e=HD + 1)[:, :, :, HD],
                ones_src)

            def a2_chunk(sc):
                # kT columns [sc*512,(sc+1)*512) and v1 s-tiles 4sc..4sc+3
                for dt_ in range(DT):
                    ps = ps_sp.tile([P, 512], F32, tag="ps2")
                    for ct in range(CT):
                        nc.tensor.matmul(
                            ps, wk_sb[:, ct, dt_ * P:(dt_ + 1) * P],
                            latT[:, ct, sc * 512:(sc + 1) * 512],
                            start=(ct == 0), stop=(ct == CT - 1))
                    nc.vector.tensor_copy(
                        kT[:, dt_, sc * 512:(sc + 1) * 512], ps)
                for s2 in range(4):
                    st = sc * 4 + s2
                    psv = ps_sp.tile([P, 512], F32, tag="ps2")
                    for ct in range(CT):
                        nc.tensor.matmul(
                            psv, latT[:, ct, st * P:(st + 1) * P],
                            wv_sb[:, ct, :],
                            start=(ct == 0), stop=(ct == CT - 1))
                    nc.vector.tensor_copy(
                        v1[:, st, :].rearrange(
                            "p (h e) -> p h e", e=HD + 1)[:, :, :HD],
                        psv.rearrange("p (h e) -> p h e", e=HD))

            def attn_block(g, ib):
                se = (2 * g) * (HD + 1)
                so = (2 * g + 1) * (HD + 1)
                ibs = slice(ib * 512, (ib + 1) * 512)
                jt_max = 4 * (ib + 1)
                # One accumulator bank per (head, contraction-half): the two
                # halves run concurrently on the T0/T8 array tiles, so they
                # must not share a PSUM bank.
                po_e0 = ps_op.tile([P, 512], F32, tag="po_e0")
                po_e1 = ps_op.tile([P, 512], F32, tag="po_e1")
                po_o0 = ps_op.tile([P, 512], F32, tag="po_o0")
                po_o1 = ps_op.tile([P, 512], F32, tag="po_o1")

                def tile_toff(jt):
                    # Diagonal-band tiles: columns < toff are fully masked,
                    # so all work is restricted to [toff, 512).  (po columns
                    # < toff then get no contribution from that j-tile, which
                    # is exactly the causal mask.)
                    return (jt - 4 * ib) * P if jt >= 4 * ib else 0

                def issue_scores(jt):
                    # both heads' scores in one 2-bank PSUM tile so a single
                    # exp + mask instruction covers the pair
                    toff = tile_toff(jt)
                    jts = slice(jt * P, (jt + 1) * P)
                    iw = slice(ib * 512 + toff, (ib + 1) * 512)
                    ps2 = ps_sp.tile([P, 2, 512], F32, tag="ps2")
                    nc.tensor.matmul(
                        ps2[:, 0, toff:], kT[0:HD, g, jts],
                        qT[0:HD, g, iw], start=True, stop=True)
                    nc.tensor.matmul(
                        ps2[:, 1, toff:], kT[HD:P, g, jts],
                        qT[HD:P, g, iw], start=True, stop=True)
                    return ps2

                ps2_next = issue_scores(0)
                for jt in range(jt_max):
                    ps2 = ps2_next
                    # issue next tile's scores ahead of this tile's AV so the
                    # PE never idles on the exp (nor the ACT on fresh scores)
                    if jt + 1 < jt_max:
                        ps2_next = issue_scores(jt + 1)
                    toff = tile_toff(jt)
                    w = 512 - toff
                    pt2 = ptp.tile([P, 2, 512], BF16, tag="pt2")
                    nc.scalar.activation(
                        pt2[:, :, toff:], ps2[:, :, toff:],
                        mybir.ActivationFunctionType.Exp, scale=SCALE)
                    if toff or jt >= 4 * ib:  # diagonal band: causal mask
                        nc.vector.tensor_mul(
                            out=pt2[:, :, toff:], in0=pt2[:, :, toff:],
                            in1=bigmask[:, 384:896 - toff]
                            .rearrange("p (o f) -> p o f", o=1)
                            .broadcast_to((P, 2, w)))
                    last = jt == jt_max - 1
                    nc.tensor.matmul(
                        po_e0[:HD + 1, toff:], v1[0:HD, jt, se:se + HD + 1],
                        pt2[0:HD, 0, toff:], start=(jt == 0), stop=last)
                    nc.tensor.matmul(
                        po_o0[:HD + 1, toff:], v1[0:HD, jt, so:so + HD + 1],
                        pt2[0:HD, 1, toff:], start=(jt == 0), stop=last)
                    nc.tensor.matmul(
                        po_e1[:HD + 1, toff:], v1[HD:P, jt, se:se + HD + 1],
                        pt2[HD:P, 0, toff:], start=(jt == 0), stop=last)
                    nc.tensor.matmul(
                        po_o1[:HD + 1, toff:], v1[HD:P, jt, so:so + HD + 1],
                        pt2[HD:P, 1, toff:], start=(jt == 0), stop=last)
                for po0, po1, pbase in ((po_e0, po_e1, 0),
                                        (po_o0, po_o1, HD)):
                    # DVE may read only one PSUM input per op
                    pm = smallp.tile([HD + 1, 512], F32, tag="pm")
                    nc.vector.tensor_copy(pm, po1[:HD + 1, :])
                    nc.vector.tensor_add(
                        out=pm, in0=po0[:HD + 1, :], in1=pm)
                    dn = smallp.tile([1, 512], F32, tag="dn")
                    # reciprocal_approx_fast mis-reads inputs at a nonzero
                    # base partition; stage the denominator at partition 0
                    nc.vector.tensor_copy(dn, pm[HD:HD + 1, :])
                    rc = smallp.tile([1, 512], F32, tag="rc")
                    nc.vector.reciprocal_approx_fast(out=rc, in_=dn)
                    rc64 = smallp.tile([HD, 512], F32, tag="rc64")
                    nc.gpsimd.partition_broadcast(rc64, rc)
                    nc.vector.tensor_mul(
                        out=OT[pbase:pbase + HD, g, ibs],
                        in0=pm[:HD, :], in1=rc64)

            def cproj_tile(m):
                osb = osbp.tile([P, H], F32, tag="osb")
                for n in range(H // 512):
                    pc = ps_sp.tile([P, 512], F32, tag="ps2")
                    for kt in range(DT):
                        nc.tensor.matmul(
                            pc, OT[:, kt, m * P:(m + 1) * P],
                            wo_sb[:, kt, n * 512:(n + 1) * 512],
                            start=(kt == 0), stop=(kt == DT - 1))
                    # split PSUM evictions between DVE and ACT
                    if n % 2 == 0:
                        nc.vector.tensor_copy(
                            osb[:, n * 512:(n + 1) * 512], pc)
                    else:
                        nc.scalar.copy(osb[:, n * 512:(n + 1) * 512], pc)
                nc.sync.dma_start(out[m * P:(m + 1) * P, :], osb)

            for ib in range(4):
                a2_chunk(ib)
                for g in range(DT):
                    attn_block(g, ib)
                for m in range(4 * ib, 4 * ib + 4):
                    cproj_tile(m)

    nc.compile()
    return nc


def _get_program():
    if "nc" not in _CACHE:
        _CACHE["nc"] = build_program()
    return _CACHE["nc"]


def make_in_maps(hidden_states, wq, w_kv_down, w_k_up, w_v_up, w_out):
    """Host-side sharding: core i -> (batch i//2, head-half i%2)."""
    in_maps = []
    for i in range(8):
        b, hp = i // 2, i % 2
        sl = slice(hp * DL, (hp + 1) * DL)
        in_maps.append({
            "hs": np.ascontiguousarray(hidden_states[b]),
            "wq": np.ascontiguousarray(wq[:, sl]),
            "wkv": np.ascontiguousarray(w_kv_down),
            "wk": np.ascontiguousarray(w_k_up[:, sl]),
            "wv": np.ascontiguousarray(w_v_up[:, sl]),
            "wo": np.ascontiguousarray(w_out[sl, :]),
        })
    return in_maps


def gather(results):
    """Host-side unshard: sum the two head-half partials per batch."""
    out = np.empty((B, S, H), dtype=np.float32)
    for b in range(B):
        out[b] = results[2 * b]["out"] + results[2 * b + 1]["out"]
    return out


def kernel(hidden_states, wq, w_kv_down, w_k_up, w_v_up, w_out, _trace=False):
    nc = _get_program()
    in_maps = make_in_maps(
        np.asarray(hidden_states, dtype=np.float32),
        np.asarray(wq, dtype=np.float32),
        np.asarray(w_kv_down, dtype=np.float32),
        np.asarray(w_k_up, dtype=np.float32),
        np.asarray(w_v_up, dtype=np.float32),
        np.asarray(w_out, dtype=np.float32),
    )
    res = run_bass_kernel_spmd(nc, in_maps, list(range(8)), trace=_trace)
    out = gather(res.results)
    if _trace:
        return out, res
    return out


# revision 31
# speedup vs baseline: 1.0166x; 1.0166x over previous
"""MLA-style latent attention kernel for Trainium2, 8-core SPMD.

Problem: B=4, S=2048, H=2048, NH=16, HD=64, KVC=512, causal softmax attention.

Sharding: core i handles batch b = i//2 and head-half hp = i%2 (8 heads).
Each core computes its partial c_proj output (contraction over its 512 of the
1024 attn-out dims); the host sums the two partials per batch.

Per-core dataflow (all on one NeuronCore, fp32 I/O, fp32r matmuls):
  hidden [S,H] --PE transpose--> hiddenT [H,S] (streamed in s-chunks)
  qT   [512,S] = wq_l^T   @ hiddenT   (lhsT=wq_l,  rhs=hiddenT)
  latT [512,S] = wkv^T    @ hiddenT
  kT   [512,S] = wk_l^T   @ latT
  v    [S,512] (natural; stored as v1 [S, 8*(64+1)] with a ones column per
               head so the AV matmul also produces the softmax denominator)
  per head h, per 512-query block:
    S^T tile [128j, 512i] = kT_h-block^T(stationary) x qT_h-block(moving)
    P^T = exp(0.125*S^T) * causal-mask      (ACT exp, DVE mask-mul)
    [O^T; denom] += [v_h | 1]^T x P^T       (PSUM accumulate over j-tiles)
    OT_h = O^T * (1/denom)                  (DVE reciprocal + bcast mul)
  out [S,H] partial = OT^T(stationary per s-tile) x wo_l(moving)
"""

import os
import sys

import numpy as np

for _p in ("/opt/trn_rl_repo",):
    if os.path.isdir(_p) and _p not in sys.path:
        sys.path.append(_p)

import concourse.bass as bass  # noqa: E402
import concourse.mybir as mybir  # noqa: E402
from concourse import bacc, tile  # noqa: E402
from concourse.bass_utils import run_bass_kernel_spmd  # noqa: E402
from concourse.masks import make_identity  # noqa: E402

F32 = mybir.dt.float32
F32R = mybir.dt.float32r
BF16 = mybir.dt.bfloat16

B, S, H = 4, 2048, 2048
NH, HD = 16, 64
KVC = 512
DL = 512          # local q/k/v dims per core (8 heads x 64)
NHL = 8           # local heads
P = 128
SCALE = 0.125

_CACHE = {}


def build_program():
    """Build + compile the per-core Bass program. Returns the Bacc module."""
    nc = bacc.Bacc("TRN2", target_bir_lowering=False, debug=False)

    hs = nc.dram_tensor("hs", [S, H], F32, kind="ExternalInput").ap()
    wq = nc.dram_tensor("wq", [H, DL], F32R, kind="ExternalInput").ap()
    wkv = nc.dram_tensor("wkv", [H, KVC], F32R, kind="ExternalInput").ap()
    wk = nc.dram_tensor("wk", [KVC, DL], F32R, kind="ExternalInput").ap()
    wv = nc.dram_tensor("wv", [KVC, DL], F32R, kind="ExternalInput").ap()
    wo = nc.dram_tensor("wo", [DL, H], F32R, kind="ExternalInput").ap()
    out = nc.dram_tensor("out", [S, H], F32, kind="ExternalOutput").ap()

    HT = H // P      # 16 h-tiles
    DT = DL // P     # 4 d-tiles
    CT = KVC // P    # 4 c-tiles
    ST = S // P      # 16 s-tiles
    SC = 256         # pass-1 s-chunk
    NCH = S // SC    # 8 chunks

    from contextlib import ExitStack

    with tile.TileContext(nc) as tc, ExitStack() as stack:
        consts = stack.enter_context(tc.tile_pool(name="consts", bufs=1))
        ident = consts.tile([P, P], F32)
        make_identity(nc, ident)
        # bigmask[j, z] = 1.0 if z - j >= 384 else 0.  The causal mask for a
        # diagonal-band tile with j0 - i0 = t is bigmask[:, 384-t : 896-t].
        bigmask = consts.tile([P, 896], BF16)
        nc.gpsimd.memset(bigmask, 1.0)
        nc.gpsimd.affine_select(
            out=bigmask, in_=bigmask,
            compare_op=mybir.AluOpType.is_ge, fill=0.0,
            base=-384, pattern=[[1, 896]], channel_multiplier=-1,
        )

        persistA = stack.enter_context(tc.tile_pool(name="persistA", bufs=1))
        qT = persistA.tile([P, DT, S], F32R, tag="qT")
        latT = persistA.tile([P, DT, S], F32R, tag="latT")

        # ---------------- phase A: hiddenT -> qT, latT ----------------
        with tc.tile_pool(name="wA", bufs=1) as wA, \
             tc.tile_pool(name="hin", bufs=2) as hinp, \
             tc.tile_pool(name="hTp", bufs=2) as hTp, \
             tc.tile_pool(name="psA_t", bufs=2, space="PSUM") as psA_t, \
             tc.tile_pool(name="psA_m", bufs=2, space="PSUM") as psA_m:
            # weights go on the gpsimd (SWDGE) queue so the first hidden
            # chunk isn't stuck behind 8MB of weight DMA on the sync queue
            wq_sb = wA.tile([P, HT, DL], F32R, tag="wq")
            nc.gpsimd.dma_start(wq_sb, wq.rearrange("(a p) d -> p a d", p=P))
            wkv_sb = wA.tile([P, HT, KVC], F32R, tag="wkv")
            nc.gpsimd.dma_start(wkv_sb, wkv.rearrange("(a p) d -> p a d", p=P))

            for c in range(NCH):
                hin = hinp.tile([P, SC // P, H], F32, tag="hin")
                nc.sync.dma_start(
                    hin, hs[c * SC:(c + 1) * SC, :].rearrange(
                        "(sp p) h -> p sp h", p=P))
                hT = hTp.tile([P, HT, SC], F32R, tag="hT")
                for sp in range(SC // P):
                    for ht4 in range(HT // 4):
                        # pack 4 transposes into one PSUM bank, evict with a
                        # single strided DVE copy
                        ptr = psA_t.tile([P, 4, P], F32, tag="ptr")
                        for k in range(4):
                            ht = ht4 * 4 + k
                            nc.tensor.transpose(
                                ptr[:, k, :], hin[:, sp, ht * P:(ht + 1) * P],
                                ident)
                        nc.vector.tensor_copy(
                            hT[:, ht4 * 4:ht4 * 4 + 4, sp * P:(sp + 1) * P],
                            ptr)
                for dt_ in range(DT):
                    ps = psA_m.tile([P, SC], F32, tag="psq")
                    for ht in range(HT):
                        nc.tensor.matmul(
                            ps, wq_sb[:, ht, dt_ * P:(dt_ + 1) * P],
                            hT[:, ht, :],
                            start=(ht == 0), stop=(ht == HT - 1))
                    nc.vector.tensor_copy(qT[:, dt_, c * SC:(c + 1) * SC], ps)
                    ps2 = psA_m.tile([P, SC], F32, tag="pslat")
                    for ht in range(HT):
                        nc.tensor.matmul(
                            ps2, wkv_sb[:, ht, dt_ * P:(dt_ + 1) * P],
                            hT[:, ht, :],
                            start=(ht == 0), stop=(ht == HT - 1))
                    nc.vector.tensor_copy(
                        latT[:, dt_, c * SC:(c + 1) * SC], ps2)

        # ------- fused phase: A2 (kT/v1) + B (attention) + C (c_proj) -------
        # Wavefront over ib: the A2 chunk sc=ib produces exactly the kT/v1
        # s-range that B's query block ib needs, and C drains the OT block
        # that B(ib) finished, so PE work (A2/C) fills the ACT-bound gaps of
        # B.  All phase-B matmuls use PE tile mode (64,128): scores have a
        # 64-deep contraction and the AV matmuls are split into two 64-deep
        # halves; a uniform mode avoids per-matmul PE reconfiguration drains,
        # and the even/odd head operands live on SBUF partitions 0-63/64-127,
        # so their matmuls run concurrently on the T0/T8 array halves.
        persistB = stack.enter_context(tc.tile_pool(name="persistB", bufs=1))
        kT = persistB.tile([P, DT, S], F32R, tag="kT")
        v1 = persistB.tile([P, ST, NHL * (HD + 1)], BF16, tag="v1")
        OT = latT  # latT[ib-block] is dead once A2(ib) ran; reuse as OT

        with tc.tile_pool(name="wB", bufs=1) as wB, \
             tc.tile_pool(name="wo_p", bufs=1) as wop, \
             tc.tile_pool(name="ptp", bufs=3) as ptp, \
             tc.tile_pool(name="smallp", bufs=2) as smallp, \
             tc.tile_pool(name="osb_p", bufs=2) as osbp, \
             tc.tile_pool(name="ps_s", bufs=2, space="PSUM") as ps_sp, \
             tc.tile_pool(name="ps_o", bufs=1, space="PSUM") as ps_op:
            wk_sb = wB.tile([P, CT, DL], F32R, tag="wk")
            nc.gpsimd.dma_start(wk_sb, wk.rearrange("(a p) d -> p a d", p=P))
            wv_sb = wB.tile([P, CT, DL], F32R, tag="wv")
            nc.gpsimd.dma_start(wv_sb, wv.rearrange("(a p) d -> p a d", p=P))
            wo_sb = wop.tile([P, DT, H], F32R, tag="wo")
            nc.gpsimd.dma_start(wo_sb, wo.rearrange("(a p) n -> p a n", p=P))
            # ones columns (column HD of each head's 65-wide slot); DVE
            # memset can't write bf16-from-imm reliably, stage in f32
            ones_src = wB.tile([P, ST, NHL], F32, tag="ones_src")
            nc.vector.memset(ones_src, 1.0)
            nc.vector.tensor_copy(
                v1.rearrange("p s (h e) -> p s h e", e=HD + 1)[:, :, :, HD],
                ones_src)

            def a2_chunk(sc):
                # kT columns [sc*512,(sc+1)*512) and v1 s-tiles 4sc..4sc+3
                for dt_ in range(DT):
                    ps = ps_op.tile([P, 512], F32, tag="po_e0")
                    for ct in range(CT):
                        nc.tensor.matmul(
                            ps, wk_sb[:, ct, dt_ * P:(dt_ + 1) * P],
                            latT[:, ct, sc * 512:(sc + 1) * 512],
                            start=(ct == 0), stop=(ct == CT - 1))
                    nc.vector.tensor_copy(
                        kT[:, dt_, sc * 512:(sc + 1) * 512], ps)
                for s2 in range(4):
                    st = sc * 4 + s2
                    psv = ps_op.tile([P, 512], F32, tag="po_o0")
                    for ct in range(CT):
                        nc.tensor.matmul(
                            psv, latT[:, ct, st * P:(st + 1) * P],
                            wv_sb[:, ct, :],
                            start=(ct == 0), stop=(ct == CT - 1))
                    nc.vector.tensor_copy(
                        v1[:, st, :].rearrange(
                            "p (h e) -> p h e", e=HD + 1)[:, :, :HD],
                        psv.rearrange("p (h e) -> p h e", e=HD))

            def attn_block(g, ib):
                se = (2 * g) * (HD + 1)
                so = (2 * g + 1) * (HD + 1)
                ibs = slice(ib * 512, (ib + 1) * 512)
                jt_max = 4 * (ib + 1)
                # One accumulator bank per (head, contraction-half): the two
                # halves run concurrently on the T0/T8 array tiles, so they
                # must not share a PSUM bank.
                po_e0 = ps_op.tile([P, 512], F32, tag="po_e0")
                po_e1 = ps_op.tile([P, 512], F32, tag="po_e1")
                po_o0 = ps_op.tile([P, 512], F32, tag="po_o0")
                po_o1 = ps_op.tile([P, 512], F32, tag="po_o1")

                def tile_toff(jt):
                    # Diagonal-band tiles: columns < toff are fully masked,
                    # so all work is restricted to [toff, 512).  (po columns
                    # < toff then get no contribution from that j-tile, which
                    # is exactly the causal mask.)
                    return (jt - 4 * ib) * P if jt >= 4 * ib else 0

                def issue_scores(jt):
                    # both heads' scores in one 2-bank PSUM tile so a single
                    # exp + mask instruction covers the pair
                    toff = tile_toff(jt)
                    jts = slice(jt * P, (jt + 1) * P)
                    iw = slice(ib * 512 + toff, (ib + 1) * 512)
                    ps2 = ps_sp.tile([P, 2, 512], F32, tag="ps2")
                    nc.tensor.matmul(
                        ps2[:, 0, toff:], kT[0:HD, g, jts],
                        qT[0:HD, g, iw], start=True, stop=True)
                    nc.tensor.matmul(
                        ps2[:, 1, toff:], kT[HD:P, g, jts],
                        qT[HD:P, g, iw], start=True, stop=True)
                    return ps2

                ps2_next = issue_scores(0)
                for jt in range(jt_max):
                    ps2 = ps2_next
                    # issue next tile's scores ahead of this tile's AV so the
                    # PE never idles on the exp (nor the ACT on fresh scores)
                    if jt + 1 < jt_max:
                        ps2_next = issue_scores(jt + 1)
                    toff = tile_toff(jt)
                    w = 512 - toff
                    pt2 = ptp.tile([P, 2, 512], BF16, tag="pt2")
                    nc.scalar.activation(
                        pt2[:, :, toff:], ps2[:, :, toff:],
                        mybir.ActivationFunctionType.Exp, scale=SCALE)
                    if toff or jt >= 4 * ib:  # diagonal band: causal mask
                        nc.vector.tensor_mul(
                            out=pt2[:, :, toff:], in0=pt2[:, :, toff:],
                            in1=bigmask[:, 384:896 - toff]
                            .rearrange("p (o f) -> p o f", o=1)
                            .broadcast_to((P, 2, w)))
                    last = jt == jt_max - 1
                    nc.tensor.matmul(
                        po_e0[:HD + 1, toff:], v1[0:HD, jt, se:se + HD + 1],
                        pt2[0:HD, 0, toff:], start=(jt == 0), stop=last)
                    nc.tensor.matmul(
                        po_o0[:HD + 1, toff:], v1[0:HD, jt, so:so + HD + 1],
                        pt2[0:HD, 1, toff:], start=(jt == 0), stop=last)
                    nc.tensor.matmul(
                        po_e1[:HD + 1, toff:], v1[HD:P, jt, se:se + HD + 1],
                        pt2[HD:P, 0, toff:], start=(jt == 0), stop=last)
                    nc.tensor.matmul(
                        po_o1[:HD + 1, toff:], v1[HD:P, jt, so:so + HD + 1],
                        pt2[HD:P, 1, toff:], start=(jt == 0), stop=last)
                for po0, po1, pbase in ((po_e0, po_e1, 0),
                                        (po_o0, po_o1, HD)):
                    # DVE may read only one PSUM input per op
                    pm = smallp.tile([HD + 1, 512], F32, tag="pm")
                    nc.vector.tensor_copy(pm, po1[:HD + 1, :])
                    nc.vector.tensor_add(
                        out=pm, in0=po0[:HD + 1, :], in1=pm)
                    dn = smallp.tile([1, 512], F32, tag="dn")
                    # reciprocal_approx_fast mis-reads inputs at a nonzero
                    # base partition; stage the denominator at partition 0
                    nc.vector.tensor_copy(dn, pm[HD:HD + 1, :])
                    rc = smallp.tile([1, 512], F32, tag="rc")
                    nc.vector.reciprocal_approx_fast(out=rc, in_=dn)
                    rc64 = smallp.tile([HD, 512], F32, tag="rc64")
                    nc.gpsimd.partition_broadcast(rc64, rc)
                    nc.vector.tensor_mul(
                        out=OT[pbase:pbase + HD, g, ibs],
                        in0=pm[:HD, :], in1=rc64)

            def cproj_tile(m):
                osb = osbp.tile([P, H], F32, tag="osb")
                for n in range(H // 512):
                    pc = ps_op.tile([P, 512], F32, tag="po_e0" if n % 2 == 0
                                    else "po_o0")
                    for kt in range(DT):
                        nc.tensor.matmul(
                            pc, OT[:, kt, m * P:(m + 1) * P],
                            wo_sb[:, kt, n * 512:(n + 1) * 512],
                            start=(kt == 0), stop=(kt == DT - 1))
                    # split PSUM evictions between DVE and ACT
                    if n % 2 == 0:
                        nc.vector.tensor_copy(
                            osb[:, n * 512:(n + 1) * 512], pc)
                    else:
                        nc.scalar.copy(osb[:, n * 512:(n + 1) * 512], pc)
                nc.sync.dma_start(out[m * P:(m + 1) * P, :], osb)

            for sc in range(4):
                a2_chunk(sc)
            for g in range(DT):
                for ib in range(4):
                    attn_block(g, ib)
            for m in range(ST):
                cproj_tile(m)

    nc.compile()
    return nc


def _get_program():
    if "nc" not in _CACHE:
        _CACHE["nc"] = build_program()
    return _CACHE["nc"]


def make_in_maps(hidden_states, wq, w_kv_down, w_k_up, w_v_up, w_out):
    """Host-side sharding: core i -> (batch i//2, head-half i%2)."""
    in_maps = []
    for i in range(8):
        b, hp = i // 2, i % 2
        sl = slice(hp * DL, (hp + 1) * DL)
        in_maps.append({
            "hs": np.ascontiguousarray(hidden_states[b]),
            "wq": np.ascontiguousarray(wq[:, sl]),
            "wkv": np.ascontiguousarray(w_kv_down),
            "wk": np.ascontiguousarray(w_k_up[:, sl]),
            "wv": np.ascontiguousarray(w_v_up[:, sl]),
            "wo": np.ascontiguousarray(w_out[sl, :]),
        })
    return in_maps


def gather(results):
    """Host-side unshard: sum the two head-half partials per batch."""
    out = np.empty((B, S, H), dtype=np.float32)
    for b in range(B):
        out[b] = results[2 * b]["out"] + results[2 * b + 1]["out"]
    return out


def kernel(hidden_states, wq, w_kv_down, w_k_up, w_v_up, w_out, _trace=False):
    nc = _get_program()
    in_maps = make_in_maps(
        np.asarray(hidden_states, dtype=np.float32),
        np.asarray(wq, dtype=np.float32),
        np.asarray(w_kv_down, dtype=np.float32),
        np.asarray(w_k_up, dtype=np.float32),
        np.asarray(w_v_up, dtype=np.float32),
        np.asarray(w_out, dtype=np.float32),
    )
    res = run_bass_kernel_spmd(nc, in_maps, list(range(8)), trace=_trace)
    out = gather(res.results)
    if _trace:
        return out, res
    return out


# revision 32
# speedup vs baseline: 1.0391x; 1.0221x over previous
"""MLA-style latent attention kernel for Trainium2, 8-core SPMD.

Problem: B=4, S=2048, H=2048, NH=16, HD=64, KVC=512, causal softmax attention.

Sharding: core i handles batch b = i//2 and head-half hp = i%2 (8 heads).
Each core computes its partial c_proj output (contraction over its 512 of the
1024 attn-out dims); the host sums the two partials per batch.

Per-core dataflow (all on one NeuronCore, fp32 I/O, fp32r matmuls):
  hidden [S,H] --PE transpose--> hiddenT [H,S] (streamed in s-chunks)
  qT   [512,S] = wq_l^T   @ hiddenT   (lhsT=wq_l,  rhs=hiddenT)
  latT [512,S] = wkv^T    @ hiddenT
  kT   [512,S] = wk_l^T   @ latT
  v    [S,512] (natural; stored as v1 [S, 8*(64+1)] with a ones column per
               head so the AV matmul also produces the softmax denominator)
  per head h, per 512-query block:
    S^T tile [128j, 512i] = kT_h-block^T(stationary) x qT_h-block(moving)
    P^T = exp(0.125*S^T) * causal-mask      (ACT exp, DVE mask-mul)
    [O^T; denom] += [v_h | 1]^T x P^T       (PSUM accumulate over j-tiles)
    OT_h = O^T * (1/denom)                  (DVE reciprocal + bcast mul)
  out [S,H] partial = OT^T(stationary per s-tile) x wo_l(moving)
"""

import os
import sys

import numpy as np

for _p in ("/opt/trn_rl_repo",):
    if os.path.isdir(_p) and _p not in sys.path:
        sys.path.append(_p)

import concourse.bass as bass  # noqa: E402
import concourse.mybir as mybir  # noqa: E402
from concourse import bacc, tile  # noqa: E402
from concourse.bass_utils import run_bass_kernel_spmd  # noqa: E402
from concourse.masks import make_identity  # noqa: E402

F32 = mybir.dt.float32
F32R = mybir.dt.float32r
BF16 = mybir.dt.bfloat16

B, S, H = 4, 2048, 2048
NH, HD = 16, 64
KVC = 512
DL = 512          # local q/k/v dims per core (8 heads x 64)
NHL = 8           # local heads
P = 128
SCALE = 0.125

_CACHE = {}


def build_program():
    """Build + compile the per-core Bass program. Returns the Bacc module."""
    nc = bacc.Bacc("TRN2", target_bir_lowering=False, debug=False)

    hs = nc.dram_tensor("hs", [S, H], F32, kind="ExternalInput").ap()
    wq = nc.dram_tensor("wq", [H, DL], F32R, kind="ExternalInput").ap()
    wkv = nc.dram_tensor("wkv", [H, KVC], F32R, kind="ExternalInput").ap()
    wk = nc.dram_tensor("wk", [KVC, DL], F32R, kind="ExternalInput").ap()
    wv = nc.dram_tensor("wv", [KVC, DL], F32R, kind="ExternalInput").ap()
    wo = nc.dram_tensor("wo", [DL, H], F32R, kind="ExternalInput").ap()
    out = nc.dram_tensor("out", [S, H], F32, kind="ExternalOutput").ap()

    HT = H // P      # 16 h-tiles
    DT = DL // P     # 4 d-tiles
    CT = KVC // P    # 4 c-tiles
    ST = S // P      # 16 s-tiles
    SC = 256         # pass-1 s-chunk
    NCH = S // SC    # 8 chunks

    from contextlib import ExitStack

    with tile.TileContext(nc) as tc, ExitStack() as stack:
        consts = stack.enter_context(tc.tile_pool(name="consts", bufs=1))
        ident = consts.tile([P, P], F32)
        make_identity(nc, ident)
        # bigmask[j, z] = 1.0 if z - j >= 384 else 0.  The causal mask for a
        # diagonal-band tile with j0 - i0 = t is bigmask[:, 384-t : 896-t].
        bigmask = consts.tile([P, 896], BF16)
        nc.gpsimd.memset(bigmask, 1.0)
        nc.gpsimd.affine_select(
            out=bigmask, in_=bigmask,
            compare_op=mybir.AluOpType.is_ge, fill=0.0,
            base=-384, pattern=[[1, 896]], channel_multiplier=-1,
        )

        persistA = stack.enter_context(tc.tile_pool(name="persistA", bufs=1))
        qT = persistA.tile([P, DT, S], F32R, tag="qT")
        latT = persistA.tile([P, DT, S], F32R, tag="latT")

        # ---------------- phase A: hiddenT -> qT, latT ----------------
        with tc.tile_pool(name="wA", bufs=1) as wA, \
             tc.tile_pool(name="hin", bufs=2) as hinp, \
             tc.tile_pool(name="hTp", bufs=2) as hTp, \
             tc.tile_pool(name="psA_t", bufs=2, space="PSUM") as psA_t, \
             tc.tile_pool(name="psA_m", bufs=2, space="PSUM") as psA_m:
            # weights go on the gpsimd (SWDGE) queue so the first hidden
            # chunk isn't stuck behind 8MB of weight DMA on the sync queue
            wq_sb = wA.tile([P, HT, DL], F32R, tag="wq")
            nc.gpsimd.dma_start(wq_sb, wq.rearrange("(a p) d -> p a d", p=P))
            wkv_sb = wA.tile([P, HT, KVC], F32R, tag="wkv")
            nc.gpsimd.dma_start(wkv_sb, wkv.rearrange("(a p) d -> p a d", p=P))

            for c in range(NCH):
                hin = hinp.tile([P, SC // P, H], F32, tag="hin")
                nc.sync.dma_start(
                    hin, hs[c * SC:(c + 1) * SC, :].rearrange(
                        "(sp p) h -> p sp h", p=P))
                hT = hTp.tile([P, HT, SC], F32R, tag="hT")
                for sp in range(SC // P):
                    for ht4 in range(HT // 4):
                        # pack 4 transposes into one PSUM bank, evict with a
                        # single strided DVE copy
                        ptr = psA_t.tile([P, 4, P], F32, tag="ptr")
                        for k in range(4):
                            ht = ht4 * 4 + k
                            nc.tensor.transpose(
                                ptr[:, k, :], hin[:, sp, ht * P:(ht + 1) * P],
                                ident)
                        nc.vector.tensor_copy(
                            hT[:, ht4 * 4:ht4 * 4 + 4, sp * P:(sp + 1) * P],
                            ptr)
                for dt_ in range(DT):
                    ps = psA_m.tile([P, SC], F32, tag="psq")
                    for ht in range(HT):
                        nc.tensor.matmul(
                            ps, wq_sb[:, ht, dt_ * P:(dt_ + 1) * P],
                            hT[:, ht, :],
                            start=(ht == 0), stop=(ht == HT - 1))
                    nc.vector.tensor_copy(qT[:, dt_, c * SC:(c + 1) * SC], ps)
                    ps2 = psA_m.tile([P, SC], F32, tag="pslat")
                    for ht in range(HT):
                        nc.tensor.matmul(
                            ps2, wkv_sb[:, ht, dt_ * P:(dt_ + 1) * P],
                            hT[:, ht, :],
                            start=(ht == 0), stop=(ht == HT - 1))
                    nc.vector.tensor_copy(
                        latT[:, dt_, c * SC:(c + 1) * SC], ps2)

        # ------- fused phase: A2 (kT/v1) + B (attention) + C (c_proj) -------
        # Wavefront over ib: the A2 chunk sc=ib produces exactly the kT/v1
        # s-range that B's query block ib needs, and C drains the OT block
        # that B(ib) finished, so PE work (A2/C) fills the ACT-bound gaps of
        # B.  All phase-B matmuls use PE tile mode (64,128): scores have a
        # 64-deep contraction and the AV matmuls are split into two 64-deep
        # halves; a uniform mode avoids per-matmul PE reconfiguration drains,
        # and the even/odd head operands live on SBUF partitions 0-63/64-127,
        # so their matmuls run concurrently on the T0/T8 array halves.
        persistB = stack.enter_context(tc.tile_pool(name="persistB", bufs=1))
        kT = persistB.tile([P, DT, S], F32R, tag="kT")
        v1 = persistB.tile([P, ST, NHL * (HD + 1)], BF16, tag="v1")
        OT = latT  # latT[ib-block] is dead once A2(ib) ran; reuse as OT

        with tc.tile_pool(name="wB", bufs=1) as wB, \
             tc.tile_pool(name="wo_p", bufs=1) as wop:
            wk_sb = wB.tile([P, CT, DL], F32R, tag="wk")
            nc.gpsimd.dma_start(wk_sb, wk.rearrange("(a p) d -> p a d", p=P))
            wv_sb = wB.tile([P, CT, DL], F32R, tag="wv")
            nc.gpsimd.dma_start(wv_sb, wv.rearrange("(a p) d -> p a d", p=P))
            wo_sb = wop.tile([P, DT, H], F32R, tag="wo")
            nc.gpsimd.dma_start(wo_sb, wo.rearrange("(a p) n -> p a n", p=P))
            # ones columns (column HD of each head's 65-wide slot); DVE
            # memset can't write bf16-from-imm reliably, stage in f32
            ones_src = wB.tile([P, ST, NHL], F32, tag="ones_src")
            nc.vector.memset(ones_src, 1.0)
            nc.vector.tensor_copy(
                v1.rearrange("p s (h e) -> p s h e", e=HD + 1)[:, :, :, HD],
                ones_src)

            def a2_chunk(sc, psB):
                # kT columns [sc*512,(sc+1)*512) and v1 s-tiles 4sc..4sc+3
                for dt_ in range(DT):
                    ps = psB.tile([P, 512], F32, tag="psk")
                    for ct in range(CT):
                        nc.tensor.matmul(
                            ps, wk_sb[:, ct, dt_ * P:(dt_ + 1) * P],
                            latT[:, ct, sc * 512:(sc + 1) * 512],
                            start=(ct == 0), stop=(ct == CT - 1))
                    nc.vector.tensor_copy(
                        kT[:, dt_, sc * 512:(sc + 1) * 512], ps)
                for s2 in range(4):
                    st = sc * 4 + s2
                    psv = psB.tile([P, 512], F32, tag="psv")
                    for ct in range(CT):
                        nc.tensor.matmul(
                            psv, latT[:, ct, st * P:(st + 1) * P],
                            wv_sb[:, ct, :],
                            start=(ct == 0), stop=(ct == CT - 1))
                    nc.vector.tensor_copy(
                        v1[:, st, :].rearrange(
                            "p (h e) -> p h e", e=HD + 1)[:, :, :HD],
                        psv.rearrange("p (h e) -> p h e", e=HD))

            def attn_block(g, ib, ps_sp, ps_op, ptp, smallp):
                se = (2 * g) * (HD + 1)
                so = (2 * g + 1) * (HD + 1)
                ibs = slice(ib * 512, (ib + 1) * 512)
                jt_max = 4 * (ib + 1)
                # One accumulator bank per (head, contraction-half): the two
                # halves run concurrently on the T0/T8 array tiles, so they
                # must not share a PSUM bank.
                po_e0 = ps_op.tile([P, 512], F32, tag="po_e0")
                po_e1 = ps_op.tile([P, 512], F32, tag="po_e1")
                po_o0 = ps_op.tile([P, 512], F32, tag="po_o0")
                po_o1 = ps_op.tile([P, 512], F32, tag="po_o1")

                def tile_toff(jt):
                    # Diagonal-band tiles: columns < toff are fully masked,
                    # so all work is restricted to [toff, 512).  (po columns
                    # < toff then get no contribution from that j-tile, which
                    # is exactly the causal mask.)
                    return (jt - 4 * ib) * P if jt >= 4 * ib else 0

                def issue_scores(jt):
                    # both heads' scores in one 2-bank PSUM tile so a single
                    # exp + mask instruction covers the pair
                    toff = tile_toff(jt)
                    jts = slice(jt * P, (jt + 1) * P)
                    iw = slice(ib * 512 + toff, (ib + 1) * 512)
                    ps2 = ps_sp.tile([P, 2, 512], F32, tag="ps2")
                    nc.tensor.matmul(
                        ps2[:, 0, toff:], kT[0:HD, g, jts],
                        qT[0:HD, g, iw], start=True, stop=True)
                    nc.tensor.matmul(
                        ps2[:, 1, toff:], kT[HD:P, g, jts],
                        qT[HD:P, g, iw], start=True, stop=True)
                    return ps2

                ps2_next = issue_scores(0)
                for jt in range(jt_max):
                    ps2 = ps2_next
                    # issue next tile's scores ahead of this tile's AV so the
                    # PE never idles on the exp (nor the ACT on fresh scores)
                    if jt + 1 < jt_max:
                        ps2_next = issue_scores(jt + 1)
                    toff = tile_toff(jt)
                    w = 512 - toff
                    pt2 = ptp.tile([P, 2, 512], BF16, tag="pt2")
                    nc.scalar.activation(
                        pt2[:, :, toff:], ps2[:, :, toff:],
                        mybir.ActivationFunctionType.Exp, scale=SCALE)
                    if toff or jt >= 4 * ib:  # diagonal band: causal mask
                        nc.vector.tensor_mul(
                            out=pt2[:, :, toff:], in0=pt2[:, :, toff:],
                            in1=bigmask[:, 384:896 - toff]
                            .rearrange("p (o f) -> p o f", o=1)
                            .broadcast_to((P, 2, w)))
                    last = jt == jt_max - 1
                    nc.tensor.matmul(
                        po_e0[:HD + 1, toff:], v1[0:HD, jt, se:se + HD + 1],
                        pt2[0:HD, 0, toff:], start=(jt == 0), stop=last)
                    nc.tensor.matmul(
                        po_o0[:HD + 1, toff:], v1[0:HD, jt, so:so + HD + 1],
                        pt2[0:HD, 1, toff:], start=(jt == 0), stop=last)
                    nc.tensor.matmul(
                        po_e1[:HD + 1, toff:], v1[HD:P, jt, se:se + HD + 1],
                        pt2[HD:P, 0, toff:], start=(jt == 0), stop=last)
                    nc.tensor.matmul(
                        po_o1[:HD + 1, toff:], v1[HD:P, jt, so:so + HD + 1],
                        pt2[HD:P, 1, toff:], start=(jt == 0), stop=last)
                for po0, po1, pbase in ((po_e0, po_e1, 0),
                                        (po_o0, po_o1, HD)):
                    # DVE may read only one PSUM input per op
                    pm = smallp.tile([HD + 1, 512], F32, tag="pm")
                    nc.vector.tensor_copy(pm, po1[:HD + 1, :])
                    nc.vector.tensor_add(
                        out=pm, in0=po0[:HD + 1, :], in1=pm)
                    dn = smallp.tile([1, 512], F32, tag="dn")
                    # reciprocal_approx_fast mis-reads inputs at a nonzero
                    # base partition; stage the denominator at partition 0
                    nc.vector.tensor_copy(dn, pm[HD:HD + 1, :])
                    rc = smallp.tile([1, 512], F32, tag="rc")
                    nc.vector.reciprocal_approx_fast(out=rc, in_=dn)
                    rc64 = smallp.tile([HD, 512], F32, tag="rc64")
                    nc.gpsimd.partition_broadcast(rc64, rc)
                    nc.vector.tensor_mul(
                        out=OT[pbase:pbase + HD, g, ibs],
                        in0=pm[:HD, :], in1=rc64)

            def cproj_tile(m, osbp, pscp):
                osb = osbp.tile([P, H], F32, tag="osb")
                for n in range(H // 512):
                    pc = pscp.tile([P, 512], F32, tag="pc")
                    for kt in range(DT):
                        nc.tensor.matmul(
                            pc, OT[:, kt, m * P:(m + 1) * P],
                            wo_sb[:, kt, n * 512:(n + 1) * 512],
                            start=(kt == 0), stop=(kt == DT - 1))
                    # split PSUM evictions between DVE and ACT
                    if n % 2 == 0:
                        nc.vector.tensor_copy(
                            osb[:, n * 512:(n + 1) * 512], pc)
                    else:
                        nc.scalar.copy(osb[:, n * 512:(n + 1) * 512], pc)
                nc.sync.dma_start(out[m * P:(m + 1) * P, :], osb)

            with tc.tile_pool(name="psB", bufs=2, space="PSUM") as psB:
                for sc in range(4):
                    a2_chunk(sc, psB)
            with tc.tile_pool(name="ptp", bufs=3) as ptp, \
                 tc.tile_pool(name="smallp", bufs=2) as smallp, \
                 tc.tile_pool(name="ps_s", bufs=2, space="PSUM") as ps_sp, \
                 tc.tile_pool(name="ps_o", bufs=1, space="PSUM") as ps_op:
                for g in range(DT):
                    for ib in range(4):
                        attn_block(g, ib, ps_sp, ps_op, ptp, smallp)
            with tc.tile_pool(name="osb_p", bufs=3) as osbp, \
                 tc.tile_pool(name="ps_c", bufs=2, space="PSUM") as pscp:
                for m in range(ST):
                    cproj_tile(m, osbp, pscp)

    nc.compile()
    return nc


def _get_program():
    if "nc" not in _CACHE:
        _CACHE["nc"] = build_program()
    return _CACHE["nc"]


def make_in_maps(hidden_states, wq, w_kv_down, w_k_up, w_v_up, w_out):
    """Host-side sharding: core i -> (batch i//2, head-half i%2)."""
    in_maps = []
    for i in range(8):
        b, hp = i // 2, i % 2
        sl = slice(hp * DL, (hp + 1) * DL)
        in_maps.append({
            "hs": np.ascontiguousarray(hidden_states[b]),
            "wq": np.ascontiguousarray(wq[:, sl]),
            "wkv": np.ascontiguousarray(w_kv_down),
            "wk": np.ascontiguousarray(w_k_up[:, sl]),
            "wv": np.ascontiguousarray(w_v_up[:, sl]),
            "wo": np.ascontiguousarray(w_out[sl, :]),
        })
    return in_maps


def gather(results):
    """Host-side unshard: sum the two head-half partials per batch."""
    out = np.empty((B, S, H), dtype=np.float32)
    for b in range(B):
        out[b] = results[2 * b]["out"] + results[2 * b + 1]["out"]
    return out


def kernel(hidden_states, wq, w_kv_down, w_k_up, w_v_up, w_out, _trace=False):
    nc = _get_program()
    in_maps = make_in_maps(
        np.asarray(hidden_states, dtype=np.float32),
        np.asarray(wq, dtype=np.float32),
        np.asarray(w_kv_down, dtype=np.float32),
        np.asarray(w_k_up, dtype=np.float32),
        np.asarray(w_v_up, dtype=np.float32),
        np.asarray(w_out, dtype=np.float32),
    )
    res = run_bass_kernel_spmd(nc, in_maps, list(range(8)), trace=_trace)
    out = gather(res.results)
    if _trace:
        return out, res
    return out


# revision 33
# speedup vs baseline: 1.0547x; 1.0150x over previous
"""MLA-style latent attention kernel for Trainium2, 8-core SPMD.

Problem: B=4, S=2048, H=2048, NH=16, HD=64, KVC=512, causal softmax attention.

Sharding: core i handles batch b = i//2 and head-half hp = i%2 (8 heads).
Each core computes its partial c_proj output (contraction over its 512 of the
1024 attn-out dims); the host sums the two partials per batch.

Per-core dataflow (all on one NeuronCore, fp32 I/O, fp32r matmuls):
  hidden [S,H] --PE transpose--> hiddenT [H,S] (streamed in s-chunks)
  qT   [512,S] = wq_l^T   @ hiddenT   (lhsT=wq_l,  rhs=hiddenT)
  latT [512,S] = wkv^T    @ hiddenT
  kT   [512,S] = wk_l^T   @ latT
  v    [S,512] (natural; stored as v1 [S, 8*(64+1)] with a ones column per
               head so the AV matmul also produces the softmax denominator)
  per head h, per 512-query block:
    S^T tile [128j, 512i] = kT_h-block^T(stationary) x qT_h-block(moving)
    P^T = exp(0.125*S^T) * causal-mask      (ACT exp, DVE mask-mul)
    [O^T; denom] += [v_h | 1]^T x P^T       (PSUM accumulate over j-tiles)
    OT_h = O^T * (1/denom)                  (DVE reciprocal + bcast mul)
  out [S,H] partial = OT^T(stationary per s-tile) x wo_l(moving)
"""

import os
import sys

import numpy as np

for _p in ("/opt/trn_rl_repo",):
    if os.path.isdir(_p) and _p not in sys.path:
        sys.path.append(_p)

import concourse.bass as bass  # noqa: E402
import concourse.mybir as mybir  # noqa: E402
from concourse import bacc, tile  # noqa: E402
from concourse.bass_utils import run_bass_kernel_spmd  # noqa: E402
from concourse.masks import make_identity  # noqa: E402

F32 = mybir.dt.float32
F32R = mybir.dt.float32r
BF16 = mybir.dt.bfloat16

B, S, H = 4, 2048, 2048
NH, HD = 16, 64
KVC = 512
DL = 512          # local q/k/v dims per core (8 heads x 64)
NHL = 8           # local heads
P = 128
SCALE = 0.125

_CACHE = {}


def build_program():
    """Build + compile the per-core Bass program. Returns the Bacc module."""
    nc = bacc.Bacc("TRN2", target_bir_lowering=False, debug=False)

    hs = nc.dram_tensor("hs", [S, H], F32, kind="ExternalInput").ap()
    wq = nc.dram_tensor("wq", [H, DL], F32R, kind="ExternalInput").ap()
    wkv = nc.dram_tensor("wkv", [H, KVC], F32R, kind="ExternalInput").ap()
    wk = nc.dram_tensor("wk", [KVC, DL], F32R, kind="ExternalInput").ap()
    wv = nc.dram_tensor("wv", [KVC, DL], F32R, kind="ExternalInput").ap()
    wo = nc.dram_tensor("wo", [DL, H], F32R, kind="ExternalInput").ap()
    out = nc.dram_tensor("out", [S, H], F32, kind="ExternalOutput").ap()

    HT = H // P      # 16 h-tiles
    DT = DL // P     # 4 d-tiles
    CT = KVC // P    # 4 c-tiles
    ST = S // P      # 16 s-tiles
    SC = 256         # pass-1 s-chunk
    NCH = S // SC    # 8 chunks

    from contextlib import ExitStack

    with tile.TileContext(nc) as tc, ExitStack() as stack:
        consts = stack.enter_context(tc.tile_pool(name="consts", bufs=1))
        ident = consts.tile([P, P], F32)
        make_identity(nc, ident)
        # bigmask[j, z] = 1.0 if z - j >= 384 else 0.  The causal mask for a
        # diagonal-band tile with j0 - i0 = t is bigmask[:, 384-t : 896-t].
        bigmask = consts.tile([P, 896], BF16)
        nc.gpsimd.memset(bigmask, 1.0)
        nc.gpsimd.affine_select(
            out=bigmask, in_=bigmask,
            compare_op=mybir.AluOpType.is_ge, fill=0.0,
            base=-384, pattern=[[1, 896]], channel_multiplier=-1,
        )

        persistA = stack.enter_context(tc.tile_pool(name="persistA", bufs=1))
        qT = persistA.tile([P, DT, S], F32R, tag="qT")
        latT = persistA.tile([P, DT, S], F32R, tag="latT")

        # ---------------- phase A: hiddenT -> qT, latT ----------------
        with tc.tile_pool(name="wA", bufs=1) as wA, \
             tc.tile_pool(name="hin", bufs=2) as hinp, \
             tc.tile_pool(name="hTp", bufs=2) as hTp, \
             tc.tile_pool(name="psA_t", bufs=2, space="PSUM") as psA_t, \
             tc.tile_pool(name="psA_m", bufs=2, space="PSUM") as psA_m:
            # weights go on the gpsimd (SWDGE) queue so the first hidden
            # chunk isn't stuck behind 8MB of weight DMA on the sync queue
            wq_sb = wA.tile([P, HT, DL], F32R, tag="wq")
            nc.gpsimd.dma_start(wq_sb, wq.rearrange("(a p) d -> p a d", p=P))
            wkv_sb = wA.tile([P, HT, KVC], F32R, tag="wkv")
            nc.gpsimd.dma_start(wkv_sb, wkv.rearrange("(a p) d -> p a d", p=P))

            for c in range(NCH):
                hin = hinp.tile([P, SC // P, H], F32, tag="hin")
                nc.sync.dma_start(
                    hin, hs[c * SC:(c + 1) * SC, :].rearrange(
                        "(sp p) h -> p sp h", p=P))
                hT = hTp.tile([P, HT, SC], F32R, tag="hT")
                for sp in range(SC // P):
                    for ht4 in range(HT // 4):
                        # pack 4 transposes into one PSUM bank, evict with a
                        # single strided DVE copy
                        ptr = psA_t.tile([P, 4, P], F32, tag="ptr")
                        for k in range(4):
                            ht = ht4 * 4 + k
                            nc.tensor.transpose(
                                ptr[:, k, :], hin[:, sp, ht * P:(ht + 1) * P],
                                ident)
                        nc.vector.tensor_copy(
                            hT[:, ht4 * 4:ht4 * 4 + 4, sp * P:(sp + 1) * P],
                            ptr)
                for dt_ in range(DT):
                    ps = psA_m.tile([P, SC], F32, tag="psq")
                    for ht in range(HT):
                        nc.tensor.matmul(
                            ps, wq_sb[:, ht, dt_ * P:(dt_ + 1) * P],
                            hT[:, ht, :],
                            start=(ht == 0), stop=(ht == HT - 1))
                    nc.vector.tensor_copy(qT[:, dt_, c * SC:(c + 1) * SC], ps)
                    ps2 = psA_m.tile([P, SC], F32, tag="pslat")
                    for ht in range(HT):
                        nc.tensor.matmul(
                            ps2, wkv_sb[:, ht, dt_ * P:(dt_ + 1) * P],
                            hT[:, ht, :],
                            start=(ht == 0), stop=(ht == HT - 1))
                    nc.vector.tensor_copy(
                        latT[:, dt_, c * SC:(c + 1) * SC], ps2)

        # ------- fused phase: A2 (kT/v1) + B (attention) + C (c_proj) -------
        # Wavefront over ib: the A2 chunk sc=ib produces exactly the kT/v1
        # s-range that B's query block ib needs, and C drains the OT block
        # that B(ib) finished, so PE work (A2/C) fills the ACT-bound gaps of
        # B.  All phase-B matmuls use PE tile mode (64,128): scores have a
        # 64-deep contraction and the AV matmuls are split into two 64-deep
        # halves; a uniform mode avoids per-matmul PE reconfiguration drains,
        # and the even/odd head operands live on SBUF partitions 0-63/64-127,
        # so their matmuls run concurrently on the T0/T8 array halves.
        persistB = stack.enter_context(tc.tile_pool(name="persistB", bufs=1))
        kT = persistB.tile([P, DT, S], F32R, tag="kT")
        v1 = persistB.tile([P, ST, NHL * (HD + 1)], BF16, tag="v1")
        OT = latT  # latT[ib-block] is dead once A2(ib) ran; reuse as OT

        with tc.tile_pool(name="wB", bufs=1) as wB, \
             tc.tile_pool(name="wo_p", bufs=1) as wop:
            wk_sb = wB.tile([P, CT, DL], F32R, tag="wk")
            nc.gpsimd.dma_start(wk_sb, wk.rearrange("(a p) d -> p a d", p=P))
            wv_sb = wB.tile([P, CT, DL], F32R, tag="wv")
            nc.gpsimd.dma_start(wv_sb, wv.rearrange("(a p) d -> p a d", p=P))
            wo_sb = wop.tile([P, DT, H], F32R, tag="wo")
            nc.gpsimd.dma_start(wo_sb, wo.rearrange("(a p) n -> p a n", p=P))
            # ones columns (column HD of each head's 65-wide slot); DVE
            # memset can't write bf16-from-imm reliably, stage in f32
            ones_src = wB.tile([P, ST, NHL], F32, tag="ones_src")
            nc.vector.memset(ones_src, 1.0)
            nc.vector.tensor_copy(
                v1.rearrange("p s (h e) -> p s h e", e=HD + 1)[:, :, :, HD],
                ones_src)

            def a2_chunk(sc, psB):
                # kT columns [sc*512,(sc+1)*512) and v1 s-tiles 4sc..4sc+3
                for dt_ in range(DT):
                    ps = psB.tile([P, 512], F32, tag="psk")
                    for ct in range(CT):
                        nc.tensor.matmul(
                            ps, wk_sb[:, ct, dt_ * P:(dt_ + 1) * P],
                            latT[:, ct, sc * 512:(sc + 1) * 512],
                            start=(ct == 0), stop=(ct == CT - 1))
                    nc.vector.tensor_copy(
                        kT[:, dt_, sc * 512:(sc + 1) * 512], ps)
                for s2 in range(4):
                    st = sc * 4 + s2
                    psv = psB.tile([P, 512], F32, tag="psv")
                    for ct in range(CT):
                        nc.tensor.matmul(
                            psv, latT[:, ct, st * P:(st + 1) * P],
                            wv_sb[:, ct, :],
                            start=(ct == 0), stop=(ct == CT - 1))
                    nc.vector.tensor_copy(
                        v1[:, st, :].rearrange(
                            "p (h e) -> p h e", e=HD + 1)[:, :, :HD],
                        psv.rearrange("p (h e) -> p h e", e=HD))

            def issue_scores(g, ib, jt, ps_sp):
                # both heads' scores in one 2-bank PSUM tile so a single
                # exp + mask instruction covers the pair.  Diagonal-band
                # tiles (toff > 0) are restricted to columns [toff, 512): the
                # columns below get no contribution from that j-tile, which
                # is exactly the causal mask.
                toff = (jt - 4 * ib) * P if jt >= 4 * ib else 0
                jts = slice(jt * P, (jt + 1) * P)
                iw = slice(ib * 512 + toff, (ib + 1) * 512)
                ps2 = ps_sp.tile([P, 2, 512], F32, tag="ps2")
                nc.tensor.matmul(
                    ps2[:, 0, toff:], kT[0:HD, g, jts],
                    qT[0:HD, g, iw], start=True, stop=True)
                nc.tensor.matmul(
                    ps2[:, 1, toff:], kT[HD:P, g, jts],
                    qT[HD:P, g, iw], start=True, stop=True)
                return ps2

            def attn_block(g, ib, ps_sp, ps_op, ptp, smallp,
                           first_ps2, next_block):
                se = (2 * g) * (HD + 1)
                so = (2 * g + 1) * (HD + 1)
                ibs = slice(ib * 512, (ib + 1) * 512)
                jt_max = 4 * (ib + 1)
                # One accumulator bank per (head, contraction-half): the two
                # halves run concurrently on the T0/T8 array tiles, so they
                # must not share a PSUM bank.
                po_e0 = ps_op.tile([P, 512], F32, tag="po_e0")
                po_e1 = ps_op.tile([P, 512], F32, tag="po_e1")
                po_o0 = ps_op.tile([P, 512], F32, tag="po_o0")
                po_o1 = ps_op.tile([P, 512], F32, tag="po_o1")

                ps2_next = first_ps2 if first_ps2 is not None \
                    else issue_scores(g, ib, 0, ps_sp)
                handoff = None
                for jt in range(jt_max):
                    ps2 = ps2_next
                    # issue the next scores ahead of this tile's AV so the PE
                    # never idles on the exp (nor the ACT on fresh scores);
                    # at the block end, prefetch the NEXT block's first tile
                    # so the ACT keeps flowing through the epilogue.
                    if jt + 1 < jt_max:
                        ps2_next = issue_scores(g, ib, jt + 1, ps_sp)
                    elif next_block is not None:
                        handoff = issue_scores(*next_block, 0, ps_sp)
                    toff = (jt - 4 * ib) * P if jt >= 4 * ib else 0
                    w = 512 - toff
                    pt2 = ptp.tile([P, 2, 512], BF16, tag="pt2")
                    nc.scalar.activation(
                        pt2[:, :, toff:], ps2[:, :, toff:],
                        mybir.ActivationFunctionType.Exp, scale=SCALE)
                    if toff or jt >= 4 * ib:  # diagonal band: causal mask
                        nc.vector.tensor_mul(
                            out=pt2[:, :, toff:], in0=pt2[:, :, toff:],
                            in1=bigmask[:, 384:896 - toff]
                            .rearrange("p (o f) -> p o f", o=1)
                            .broadcast_to((P, 2, w)))
                    last = jt == jt_max - 1
                    nc.tensor.matmul(
                        po_e0[:HD + 1, toff:], v1[0:HD, jt, se:se + HD + 1],
                        pt2[0:HD, 0, toff:], start=(jt == 0), stop=last)
                    nc.tensor.matmul(
                        po_o0[:HD + 1, toff:], v1[0:HD, jt, so:so + HD + 1],
                        pt2[0:HD, 1, toff:], start=(jt == 0), stop=last)
                    nc.tensor.matmul(
                        po_e1[:HD + 1, toff:], v1[HD:P, jt, se:se + HD + 1],
                        pt2[HD:P, 0, toff:], start=(jt == 0), stop=last)
                    nc.tensor.matmul(
                        po_o1[:HD + 1, toff:], v1[HD:P, jt, so:so + HD + 1],
                        pt2[HD:P, 1, toff:], start=(jt == 0), stop=last)
                for po0, po1, pbase in ((po_e0, po_e1, 0),
                                        (po_o0, po_o1, HD)):
                    # DVE may read only one PSUM input per op
                    pm = smallp.tile([HD + 1, 512], F32, tag="pm")
                    nc.vector.tensor_copy(pm, po1[:HD + 1, :])
                    nc.vector.tensor_add(
                        out=pm, in0=po0[:HD + 1, :], in1=pm)
                    dn = smallp.tile([1, 512], F32, tag="dn")
                    # reciprocal_approx_fast mis-reads inputs at a nonzero
                    # base partition; stage the denominator at partition 0
                    nc.vector.tensor_copy(dn, pm[HD:HD + 1, :])
                    rc = smallp.tile([1, 512], F32, tag="rc")
                    nc.vector.reciprocal_approx_fast(out=rc, in_=dn)
                    rc64 = smallp.tile([HD, 512], F32, tag="rc64")
                    nc.gpsimd.partition_broadcast(rc64, rc)
                    nc.vector.tensor_mul(
                        out=OT[pbase:pbase + HD, g, ibs],
                        in0=pm[:HD, :], in1=rc64)
                return handoff

            def cproj_tile(m, osbp, pscp):
                osb = osbp.tile([P, H], F32, tag="osb")
                for n in range(H // 512):
                    pc = pscp.tile([P, 512], F32, tag="pc")
                    for kt in range(DT):
                        nc.tensor.matmul(
                            pc, OT[:, kt, m * P:(m + 1) * P],
                            wo_sb[:, kt, n * 512:(n + 1) * 512],
                            start=(kt == 0), stop=(kt == DT - 1))
                    # split PSUM evictions between DVE and ACT
                    if n % 2 == 0:
                        nc.vector.tensor_copy(
                            osb[:, n * 512:(n + 1) * 512], pc)
                    else:
                        nc.scalar.copy(osb[:, n * 512:(n + 1) * 512], pc)
                nc.sync.dma_start(out[m * P:(m + 1) * P, :], osb)

            with tc.tile_pool(name="psB", bufs=2, space="PSUM") as psB:
                for sc in range(4):
                    a2_chunk(sc, psB)
            with tc.tile_pool(name="ptp", bufs=3) as ptp, \
                 tc.tile_pool(name="smallp", bufs=2) as smallp, \
                 tc.tile_pool(name="ps_s", bufs=2, space="PSUM") as ps_sp, \
                 tc.tile_pool(name="ps_o", bufs=1, space="PSUM") as ps_op:
                blocks = [(g, ib) for g in range(DT) for ib in range(4)]
                carry = None
                for k, (g, ib) in enumerate(blocks):
                    nxt = blocks[k + 1] if k + 1 < len(blocks) else None
                    carry = attn_block(g, ib, ps_sp, ps_op, ptp, smallp,
                                       carry, nxt)
            with tc.tile_pool(name="osb_p", bufs=3) as osbp, \
                 tc.tile_pool(name="ps_c", bufs=2, space="PSUM") as pscp:
                for m in range(ST):
                    cproj_tile(m, osbp, pscp)

    nc.compile()
    return nc


def _get_program():
    if "nc" not in _CACHE:
        _CACHE["nc"] = build_program()
    return _CACHE["nc"]


def make_in_maps(hidden_states, wq, w_kv_down, w_k_up, w_v_up, w_out):
    """Host-side sharding: core i -> (batch i//2, head-half i%2)."""
    in_maps = []
    for i in range(8):
        b, hp = i // 2, i % 2
        sl = slice(hp * DL, (hp + 1) * DL)
        in_maps.append({
            "hs": np.ascontiguousarray(hidden_states[b]),
            "wq": np.ascontiguousarray(wq[:, sl]),
            "wkv": np.ascontiguousarray(w_kv_down),
            "wk": np.ascontiguousarray(w_k_up[:, sl]),
            "wv": np.ascontiguousarray(w_v_up[:, sl]),
            "wo": np.ascontiguousarray(w_out[sl, :]),
        })
    return in_maps


def gather(results):
    """Host-side unshard: sum the two head-half partials per batch."""
    out = np.empty((B, S, H), dtype=np.float32)
    for b in range(B):
        out[b] = results[2 * b]["out"] + results[2 * b + 1]["out"]
    return out


def kernel(hidden_states, wq, w_kv_down, w_k_up, w_v_up, w_out, _trace=False):
    nc = _get_program()
    in_maps = make_in_maps(
        np.asarray(hidden_states, dtype=np.float32),
        np.asarray(wq, dtype=np.float32),
        np.asarray(w_kv_down, dtype=np.float32),
        np.asarray(w_k_up, dtype=np.float32),
        np.asarray(w_v_up, dtype=np.float32),
        np.asarray(w_out, dtype=np.float32),
    )
    res = run_bass_kernel_spmd(nc, in_maps, list(range(8)), trace=_trace)
    out = gather(res.results)
    if _trace:
        return out, res
    return out


# revision 34
# speedup vs baseline: 1.0556x; 1.0009x over previous
"""MLA-style latent attention kernel for Trainium2, 8-core SPMD.

Problem: B=4, S=2048, H=2048, NH=16, HD=64, KVC=512, causal softmax attention.

Sharding: core i handles batch b = i//2 and head-half hp = i%2 (8 heads).
Each core computes its partial c_proj output (contraction over its 512 of the
1024 attn-out dims); the host sums the two partials per batch.

Per-core dataflow (all on one NeuronCore, fp32 I/O, fp32r matmuls):
  hidden [S,H] --PE transpose--> hiddenT [H,S] (streamed in s-chunks)
  qT   [512,S] = wq_l^T   @ hiddenT   (lhsT=wq_l,  rhs=hiddenT)
  latT [512,S] = wkv^T    @ hiddenT
  kT   [512,S] = wk_l^T   @ latT
  v    [S,512] (natural; stored as v1 [S, 8*(64+1)] with a ones column per
               head so the AV matmul also produces the softmax denominator)
  per head h, per 512-query block:
    S^T tile [128j, 512i] = kT_h-block^T(stationary) x qT_h-block(moving)
    P^T = exp(0.125*S^T) * causal-mask      (ACT exp, DVE mask-mul)
    [O^T; denom] += [v_h | 1]^T x P^T       (PSUM accumulate over j-tiles)
    OT_h = O^T * (1/denom)                  (DVE reciprocal + bcast mul)
  out [S,H] partial = OT^T(stationary per s-tile) x wo_l(moving)
"""

import os
import sys

import numpy as np

for _p in ("/opt/trn_rl_repo",):
    if os.path.isdir(_p) and _p not in sys.path:
        sys.path.append(_p)

import concourse.bass as bass  # noqa: E402
import concourse.mybir as mybir  # noqa: E402
from concourse import bacc, tile  # noqa: E402
from concourse.bass_utils import run_bass_kernel_spmd  # noqa: E402
from concourse.masks import make_identity  # noqa: E402

F32 = mybir.dt.float32
F32R = mybir.dt.float32r
BF16 = mybir.dt.bfloat16

B, S, H = 4, 2048, 2048
NH, HD = 16, 64
KVC = 512
DL = 512          # local q/k/v dims per core (8 heads x 64)
NHL = 8           # local heads
P = 128
SCALE = 0.125

_CACHE = {}


def build_program():
    """Build + compile the per-core Bass program. Returns the Bacc module."""
    nc = bacc.Bacc("TRN2", target_bir_lowering=False, debug=False)

    hs = nc.dram_tensor("hs", [S, H], F32, kind="ExternalInput").ap()
    wq = nc.dram_tensor("wq", [H, DL], F32R, kind="ExternalInput").ap()
    wkv = nc.dram_tensor("wkv", [H, KVC], F32R, kind="ExternalInput").ap()
    wk = nc.dram_tensor("wk", [KVC, DL], F32R, kind="ExternalInput").ap()
    wv = nc.dram_tensor("wv", [KVC, DL], F32R, kind="ExternalInput").ap()
    wo = nc.dram_tensor("wo", [DL, H], F32R, kind="ExternalInput").ap()
    out = nc.dram_tensor("out", [S, H], F32, kind="ExternalOutput").ap()

    HT = H // P      # 16 h-tiles
    DT = DL // P     # 4 d-tiles
    CT = KVC // P    # 4 c-tiles
    ST = S // P      # 16 s-tiles
    SC = 256         # pass-1 s-chunk
    NCH = S // SC    # 8 chunks

    from contextlib import ExitStack

    with tile.TileContext(nc) as tc, ExitStack() as stack:
        consts = stack.enter_context(tc.tile_pool(name="consts", bufs=1))
        ident = consts.tile([P, P], F32)
        make_identity(nc, ident)
        # bigmask[j, z] = 1.0 if z - j >= 384 else 0.  The causal mask for a
        # diagonal-band tile with j0 - i0 = t is bigmask[:, 384-t : 896-t].
        bigmask = consts.tile([P, 896], BF16)
        nc.gpsimd.memset(bigmask, 1.0)
        nc.gpsimd.affine_select(
            out=bigmask, in_=bigmask,
            compare_op=mybir.AluOpType.is_ge, fill=0.0,
            base=-384, pattern=[[1, 896]], channel_multiplier=-1,
        )

        persistA = stack.enter_context(tc.tile_pool(name="persistA", bufs=1))
        qT = persistA.tile([P, DT, S], F32R, tag="qT")
        latT = persistA.tile([P, DT, S], F32R, tag="latT")

        # ---------------- phase A: hiddenT -> qT, latT ----------------
        with tc.tile_pool(name="wA", bufs=1) as wA, \
             tc.tile_pool(name="hin", bufs=2) as hinp, \
             tc.tile_pool(name="hTp", bufs=2) as hTp, \
             tc.tile_pool(name="psA_t", bufs=2, space="PSUM") as psA_t, \
             tc.tile_pool(name="psA_m", bufs=2, space="PSUM") as psA_m:
            # weights go on the gpsimd (SWDGE) queue so the first hidden
            # chunk isn't stuck behind 8MB of weight DMA on the sync queue
            wq_sb = wA.tile([P, HT, DL], F32R, tag="wq")
            nc.gpsimd.dma_start(wq_sb, wq.rearrange("(a p) d -> p a d", p=P))
            wkv_sb = wA.tile([P, HT, KVC], F32R, tag="wkv")
            nc.gpsimd.dma_start(wkv_sb, wkv.rearrange("(a p) d -> p a d", p=P))

            for c in range(NCH):
                hin = hinp.tile([P, SC // P, H], F32, tag="hin")
                nc.sync.dma_start(
                    hin, hs[c * SC:(c + 1) * SC, :].rearrange(
                        "(sp p) h -> p sp h", p=P))
                hT = hTp.tile([P, HT, SC], F32R, tag="hT")
                for sp in range(SC // P):
                    for ht4 in range(HT // 4):
                        # pack 4 transposes into one PSUM bank, evict with a
                        # single strided DVE copy
                        ptr = psA_t.tile([P, 4, P], F32, tag="ptr")
                        for k in range(4):
                            ht = ht4 * 4 + k
                            nc.tensor.transpose(
                                ptr[:, k, :], hin[:, sp, ht * P:(ht + 1) * P],
                                ident)
                        nc.vector.tensor_copy(
                            hT[:, ht4 * 4:ht4 * 4 + 4, sp * P:(sp + 1) * P],
                            ptr)
                for dt_ in range(DT):
                    ps = psA_m.tile([P, SC], F32, tag="psq")
                    for ht in range(HT):
                        nc.tensor.matmul(
                            ps, wq_sb[:, ht, dt_ * P:(dt_ + 1) * P],
                            hT[:, ht, :],
                            start=(ht == 0), stop=(ht == HT - 1))
                    nc.vector.tensor_copy(qT[:, dt_, c * SC:(c + 1) * SC], ps)
                    ps2 = psA_m.tile([P, SC], F32, tag="pslat")
                    for ht in range(HT):
                        nc.tensor.matmul(
                            ps2, wkv_sb[:, ht, dt_ * P:(dt_ + 1) * P],
                            hT[:, ht, :],
                            start=(ht == 0), stop=(ht == HT - 1))
                    nc.vector.tensor_copy(
                        latT[:, dt_, c * SC:(c + 1) * SC], ps2)

        # ------- fused phase: A2 (kT/v1) + B (attention) + C (c_proj) -------
        # Wavefront over ib: the A2 chunk sc=ib produces exactly the kT/v1
        # s-range that B's query block ib needs, and C drains the OT block
        # that B(ib) finished, so PE work (A2/C) fills the ACT-bound gaps of
        # B.  All phase-B matmuls use PE tile mode (64,128): scores have a
        # 64-deep contraction and the AV matmuls are split into two 64-deep
        # halves; a uniform mode avoids per-matmul PE reconfiguration drains,
        # and the even/odd head operands live on SBUF partitions 0-63/64-127,
        # so their matmuls run concurrently on the T0/T8 array halves.
        persistB = stack.enter_context(tc.tile_pool(name="persistB", bufs=1))
        kT = persistB.tile([P, DT, S], F32R, tag="kT")
        v1 = persistB.tile([P, ST, NHL * (HD + 1)], BF16, tag="v1")
        OT = latT  # latT[ib-block] is dead once A2(ib) ran; reuse as OT

        with tc.tile_pool(name="wB", bufs=1) as wB, \
             tc.tile_pool(name="wo_p", bufs=1) as wop:
            wk_sb = wB.tile([P, CT, DL], F32R, tag="wk")
            nc.gpsimd.dma_start(wk_sb, wk.rearrange("(a p) d -> p a d", p=P))
            wv_sb = wB.tile([P, CT, DL], F32R, tag="wv")
            nc.gpsimd.dma_start(wv_sb, wv.rearrange("(a p) d -> p a d", p=P))
            wo_sb = wop.tile([P, DT, H], F32R, tag="wo")
            nc.gpsimd.dma_start(wo_sb, wo.rearrange("(a p) n -> p a n", p=P))
            # ones columns (column HD of each head's 65-wide slot); DVE
            # memset can't write bf16-from-imm reliably, stage in f32
            ones_src = wB.tile([P, ST, NHL], F32, tag="ones_src")
            nc.vector.memset(ones_src, 1.0)
            nc.vector.tensor_copy(
                v1.rearrange("p s (h e) -> p s h e", e=HD + 1)[:, :, :, HD],
                ones_src)

            def a2_chunk(sc, psB):
                # kT columns [sc*512,(sc+1)*512) and v1 s-tiles 4sc..4sc+3
                for dt_ in range(DT):
                    ps = psB.tile([P, 512], F32, tag="psk")
                    for ct in range(CT):
                        nc.tensor.matmul(
                            ps, wk_sb[:, ct, dt_ * P:(dt_ + 1) * P],
                            latT[:, ct, sc * 512:(sc + 1) * 512],
                            start=(ct == 0), stop=(ct == CT - 1))
                    nc.vector.tensor_copy(
                        kT[:, dt_, sc * 512:(sc + 1) * 512], ps)
                for s2 in range(4):
                    st = sc * 4 + s2
                    psv = psB.tile([P, 512], F32, tag="psv")
                    for ct in range(CT):
                        nc.tensor.matmul(
                            psv, latT[:, ct, st * P:(st + 1) * P],
                            wv_sb[:, ct, :],
                            start=(ct == 0), stop=(ct == CT - 1))
                    nc.vector.tensor_copy(
                        v1[:, st, :].rearrange(
                            "p (h e) -> p h e", e=HD + 1)[:, :, :HD],
                        psv.rearrange("p (h e) -> p h e", e=HD))

            def issue_scores(g, ib, jt, ps_sp):
                # both heads' scores in one 2-bank PSUM tile so a single
                # exp + mask instruction covers the pair.  Diagonal-band
                # tiles (toff > 0) are restricted to columns [toff, 512): the
                # columns below get no contribution from that j-tile, which
                # is exactly the causal mask.
                toff = (jt - 4 * ib) * P if jt >= 4 * ib else 0
                jts = slice(jt * P, (jt + 1) * P)
                iw = slice(ib * 512 + toff, (ib + 1) * 512)
                ps2 = ps_sp.tile([P, 2, 512], F32, tag="ps2")
                nc.tensor.matmul(
                    ps2[:, 0, toff:], kT[0:HD, g, jts],
                    qT[0:HD, g, iw], start=True, stop=True)
                nc.tensor.matmul(
                    ps2[:, 1, toff:], kT[HD:P, g, jts],
                    qT[HD:P, g, iw], start=True, stop=True)
                return ps2

            def attn_block(g, ib, ps_sp, ps_op, ptp, smallp,
                           first_ps2, next_block):
                se = (2 * g) * (HD + 1)
                so = (2 * g + 1) * (HD + 1)
                ibs = slice(ib * 512, (ib + 1) * 512)
                jt_max = 4 * (ib + 1)
                # One accumulator bank per (head, contraction-half): the two
                # halves run concurrently on the T0/T8 array tiles, so they
                # must not share a PSUM bank.
                po_e0 = ps_op.tile([P, 512], F32, tag="po_e0")
                po_e1 = ps_op.tile([P, 512], F32, tag="po_e1")
                po_o0 = ps_op.tile([P, 512], F32, tag="po_o0")
                po_o1 = ps_op.tile([P, 512], F32, tag="po_o1")

                ps2_next = first_ps2 if first_ps2 is not None \
                    else issue_scores(g, ib, 0, ps_sp)
                handoff = None
                for jt in range(jt_max):
                    ps2 = ps2_next
                    # issue the next scores ahead of this tile's AV so the PE
                    # never idles on the exp (nor the ACT on fresh scores);
                    # at the block end, prefetch the NEXT block's first tile
                    # so the ACT keeps flowing through the epilogue.
                    if jt + 1 < jt_max:
                        ps2_next = issue_scores(g, ib, jt + 1, ps_sp)
                    elif next_block is not None:
                        handoff = issue_scores(*next_block, 0, ps_sp)
                    toff = (jt - 4 * ib) * P if jt >= 4 * ib else 0
                    w = 512 - toff
                    pt2 = ptp.tile([P, 2, 512], BF16, tag="pt2")
                    nc.scalar.activation(
                        pt2[:, :, toff:], ps2[:, :, toff:],
                        mybir.ActivationFunctionType.Exp, scale=SCALE)
                    if jt >= 4 * ib:  # diagonal band: causal mask.  Only
                        # the 128-wide triangle band [toff, toff+128) can
                        # contain masked elements; later columns are fully
                        # below the diagonal.
                        mw = min(P, w)
                        nc.vector.tensor_mul(
                            out=pt2[:, :, toff:toff + mw],
                            in0=pt2[:, :, toff:toff + mw],
                            in1=bigmask[:, 384:384 + mw]
                            .rearrange("p (o f) -> p o f", o=1)
                            .broadcast_to((P, 2, mw)))
                    last = jt == jt_max - 1
                    nc.tensor.matmul(
                        po_e0[:HD + 1, toff:], v1[0:HD, jt, se:se + HD + 1],
                        pt2[0:HD, 0, toff:], start=(jt == 0), stop=last)
                    nc.tensor.matmul(
                        po_o0[:HD + 1, toff:], v1[0:HD, jt, so:so + HD + 1],
                        pt2[0:HD, 1, toff:], start=(jt == 0), stop=last)
                    nc.tensor.matmul(
                        po_e1[:HD + 1, toff:], v1[HD:P, jt, se:se + HD + 1],
                        pt2[HD:P, 0, toff:], start=(jt == 0), stop=last)
                    nc.tensor.matmul(
                        po_o1[:HD + 1, toff:], v1[HD:P, jt, so:so + HD + 1],
                        pt2[HD:P, 1, toff:], start=(jt == 0), stop=last)
                for po0, po1, pbase in ((po_e0, po_e1, 0),
                                        (po_o0, po_o1, HD)):
                    # DVE may read only one PSUM input per op
                    pm = smallp.tile([HD + 1, 512], F32, tag="pm")
                    nc.vector.tensor_copy(pm, po1[:HD + 1, :])
                    nc.vector.tensor_add(
                        out=pm, in0=po0[:HD + 1, :], in1=pm)
                    dn = smallp.tile([1, 512], F32, tag="dn")
                    # reciprocal_approx_fast mis-reads inputs at a nonzero
                    # base partition; stage the denominator at partition 0
                    nc.vector.tensor_copy(dn, pm[HD:HD + 1, :])
                    rc = smallp.tile([1, 512], F32, tag="rc")
                    nc.vector.reciprocal_approx_fast(out=rc, in_=dn)
                    rc64 = smallp.tile([HD, 512], F32, tag="rc64")
                    nc.gpsimd.partition_broadcast(rc64, rc)
                    nc.vector.tensor_mul(
                        out=OT[pbase:pbase + HD, g, ibs],
                        in0=pm[:HD, :], in1=rc64)
                return handoff

            def cproj_tile(m, osbp, pscp):
                osb = osbp.tile([P, H], F32, tag="osb")
                for n in range(H // 512):
                    pc = pscp.tile([P, 512], F32, tag="pc")
                    for kt in range(DT):
                        nc.tensor.matmul(
                            pc, OT[:, kt, m * P:(m + 1) * P],
                            wo_sb[:, kt, n * 512:(n + 1) * 512],
                            start=(kt == 0), stop=(kt == DT - 1))
                    # split PSUM evictions between DVE and ACT
                    if n % 2 == 0:
                        nc.vector.tensor_copy(
                            osb[:, n * 512:(n + 1) * 512], pc)
                    else:
                        nc.scalar.copy(osb[:, n * 512:(n + 1) * 512], pc)
                nc.sync.dma_start(out[m * P:(m + 1) * P, :], osb)

            with tc.tile_pool(name="psB", bufs=2, space="PSUM") as psB:
                for sc in range(4):
                    a2_chunk(sc, psB)
            with tc.tile_pool(name="ptp", bufs=3) as ptp, \
                 tc.tile_pool(name="smallp", bufs=2) as smallp, \
                 tc.tile_pool(name="ps_s", bufs=2, space="PSUM") as ps_sp, \
                 tc.tile_pool(name="ps_o", bufs=1, space="PSUM") as ps_op:
                blocks = [(g, ib) for g in range(DT) for ib in range(4)]
                carry = None
                for k, (g, ib) in enumerate(blocks):
                    nxt = blocks[k + 1] if k + 1 < len(blocks) else None
                    carry = attn_block(g, ib, ps_sp, ps_op, ptp, smallp,
                                       carry, nxt)
            with tc.tile_pool(name="osb_p", bufs=3) as osbp, \
                 tc.tile_pool(name="ps_c", bufs=2, space="PSUM") as pscp:
                for m in range(ST):
                    cproj_tile(m, osbp, pscp)

    nc.compile()
    return nc


def _get_program():
    if "nc" not in _CACHE:
        _CACHE["nc"] = build_program()
    return _CACHE["nc"]


def make_in_maps(hidden_states, wq, w_kv_down, w_k_up, w_v_up, w_out):
    """Host-side sharding: core i -> (batch i//2, head-half i%2)."""
    in_maps = []
    for i in range(8):
        b, hp = i // 2, i % 2
        sl = slice(hp * DL, (hp + 1) * DL)
        in_maps.append({
            "hs": np.ascontiguousarray(hidden_states[b]),
            "wq": np.ascontiguousarray(wq[:, sl]),
            "wkv": np.ascontiguousarray(w_kv_down),
            "wk": np.ascontiguousarray(w_k_up[:, sl]),
            "wv": np.ascontiguousarray(w_v_up[:, sl]),
            "wo": np.ascontiguousarray(w_out[sl, :]),
        })
    return in_maps


def gather(results):
    """Host-side unshard: sum the two head-half partials per batch."""
    out = np.empty((B, S, H), dtype=np.float32)
    for b in range(B):
        out[b] = results[2 * b]["out"] + results[2 * b + 1]["out"]
    return out


def kernel(hidden_states, wq, w_kv_down, w_k_up, w_v_up, w_out, _trace=False):
    nc = _get_program()
    in_maps = make_in_maps(
        np.asarray(hidden_states, dtype=np.float32),
        np.asarray(wq, dtype=np.float32),
        np.asarray(w_kv_down, dtype=np.float32),
        np.asarray(w_k_up, dtype=np.float32),
        np.asarray(w_v_up, dtype=np.float32),
        np.asarray(w_out, dtype=np.float32),
    )
    res = run_bass_kernel_spmd(nc, in_maps, list(range(8)), trace=_trace)
    out = gather(res.results)
    if _trace:
        return out, res
    return out


# revision 35
# speedup vs baseline: 1.0707x; 1.0143x over previous
"""MLA-style latent attention kernel for Trainium2, 8-core SPMD.

Problem: B=4, S=2048, H=2048, NH=16, HD=64, KVC=512, causal softmax attention.

Sharding: core i handles batch b = i//2 and head-half hp = i%2 (8 heads).
Each core computes its partial c_proj output (contraction over its 512 of the
1024 attn-out dims); the host sums the two partials per batch.

Per-core dataflow (all on one NeuronCore, fp32 I/O, fp32r matmuls):
  hidden [S,H] --PE transpose--> hiddenT [H,S] (streamed in s-chunks)
  qT   [512,S] = wq_l^T   @ hiddenT   (lhsT=wq_l,  rhs=hiddenT)
  latT [512,S] = wkv^T    @ hiddenT
  kT   [512,S] = wk_l^T   @ latT
  v    [S,512] (natural; stored as v1 [S, 8*(64+1)] with a ones column per
               head so the AV matmul also produces the softmax denominator)
  per head h, per 512-query block:
    S^T tile [128j, 512i] = kT_h-block^T(stationary) x qT_h-block(moving)
    P^T = exp(0.125*S^T) * causal-mask      (ACT exp, DVE mask-mul)
    [O^T; denom] += [v_h | 1]^T x P^T       (PSUM accumulate over j-tiles)
    OT_h = O^T * (1/denom)                  (DVE reciprocal + bcast mul)
  out [S,H] partial = OT^T(stationary per s-tile) x wo_l(moving)
"""

import os
import sys

import numpy as np

for _p in ("/opt/trn_rl_repo",):
    if os.path.isdir(_p) and _p not in sys.path:
        sys.path.append(_p)

import concourse.bass as bass  # noqa: E402
import concourse.mybir as mybir  # noqa: E402
from concourse import bacc, tile  # noqa: E402
from concourse.bass_utils import run_bass_kernel_spmd  # noqa: E402
from concourse.masks import make_identity  # noqa: E402

F32 = mybir.dt.float32
F32R = mybir.dt.float32r
BF16 = mybir.dt.bfloat16

B, S, H = 4, 2048, 2048
NH, HD = 16, 64
KVC = 512
DL = 512          # local q/k/v dims per core (8 heads x 64)
NHL = 8           # local heads
P = 128
SCALE = 0.125

_CACHE = {}


def build_program():
    """Build + compile the per-core Bass program. Returns the Bacc module."""
    nc = bacc.Bacc("TRN2", target_bir_lowering=False, debug=False)

    hs = nc.dram_tensor("hs", [S, H], F32, kind="ExternalInput").ap()
    wq = nc.dram_tensor("wq", [H, DL], F32R, kind="ExternalInput").ap()
    wkv = nc.dram_tensor("wkv", [H, KVC], F32R, kind="ExternalInput").ap()
    wk = nc.dram_tensor("wk", [KVC, DL], F32R, kind="ExternalInput").ap()
    wv = nc.dram_tensor("wv", [KVC, DL], F32R, kind="ExternalInput").ap()
    wo = nc.dram_tensor("wo", [DL, H], F32R, kind="ExternalInput").ap()
    out = nc.dram_tensor("out", [S, H], F32, kind="ExternalOutput").ap()

    HT = H // P      # 16 h-tiles
    DT = DL // P     # 4 d-tiles
    CT = KVC // P    # 4 c-tiles
    ST = S // P      # 16 s-tiles
    SC = 256         # pass-1 s-chunk
    NCH = S // SC    # 8 chunks

    from contextlib import ExitStack

    with tile.TileContext(nc) as tc, ExitStack() as stack:
        consts = stack.enter_context(tc.tile_pool(name="consts", bufs=1))
        ident = consts.tile([P, P], F32)
        make_identity(nc, ident)
        # bigmask[j, z] = 1.0 if z - j >= 384 else 0.  The causal mask for a
        # diagonal-band tile with j0 - i0 = t is bigmask[:, 384-t : 896-t].
        bigmask = consts.tile([P, 896], BF16)
        nc.gpsimd.memset(bigmask, 1.0)
        nc.gpsimd.affine_select(
            out=bigmask, in_=bigmask,
            compare_op=mybir.AluOpType.is_ge, fill=0.0,
            base=-384, pattern=[[1, 896]], channel_multiplier=-1,
        )

        persistA = stack.enter_context(tc.tile_pool(name="persistA", bufs=1))
        qT = persistA.tile([P, DT, S], F32R, tag="qT")
        latT = persistA.tile([P, DT, S], F32R, tag="latT")

        # ---------------- phase A: hiddenT -> qT, latT ----------------
        with tc.tile_pool(name="wA", bufs=1) as wA, \
             tc.tile_pool(name="hin", bufs=2) as hinp, \
             tc.tile_pool(name="hTp", bufs=2) as hTp, \
             tc.tile_pool(name="psA_t", bufs=2, space="PSUM") as psA_t, \
             tc.tile_pool(name="psA_m", bufs=2, space="PSUM") as psA_m:
            # DMA order on the fast HWDGE (sync) queue is the phase-A
            # critical path: wq first (first qT chain needs it), then the
            # first hidden chunk, then wkv, then the remaining chunks.
            wq_sb = wA.tile([P, HT, DL], F32R, tag="wq")
            nc.sync.dma_start(wq_sb, wq.rearrange("(a p) d -> p a d", p=P))
            wkv_sb = wA.tile([P, HT, KVC], F32R, tag="wkv")

            for c in range(NCH):
                hin = hinp.tile([P, SC // P, H], F32, tag="hin")
                nc.sync.dma_start(
                    hin, hs[c * SC:(c + 1) * SC, :].rearrange(
                        "(sp p) h -> p sp h", p=P))
                if c == 0:
                    nc.sync.dma_start(
                        wkv_sb, wkv.rearrange("(a p) d -> p a d", p=P))
                hT = hTp.tile([P, HT, SC], F32R, tag="hT")
                for sp in range(SC // P):
                    for ht4 in range(HT // 4):
                        # pack 4 transposes into one PSUM bank, evict with a
                        # single strided DVE copy
                        ptr = psA_t.tile([P, 4, P], F32, tag="ptr")
                        for k in range(4):
                            ht = ht4 * 4 + k
                            nc.tensor.transpose(
                                ptr[:, k, :], hin[:, sp, ht * P:(ht + 1) * P],
                                ident)
                        nc.vector.tensor_copy(
                            hT[:, ht4 * 4:ht4 * 4 + 4, sp * P:(sp + 1) * P],
                            ptr)
                for dt_ in range(DT):
                    ps = psA_m.tile([P, SC], F32, tag="psq")
                    for ht in range(HT):
                        nc.tensor.matmul(
                            ps, wq_sb[:, ht, dt_ * P:(dt_ + 1) * P],
                            hT[:, ht, :],
                            start=(ht == 0), stop=(ht == HT - 1))
                    nc.vector.tensor_copy(qT[:, dt_, c * SC:(c + 1) * SC], ps)
                    ps2 = psA_m.tile([P, SC], F32, tag="pslat")
                    for ht in range(HT):
                        nc.tensor.matmul(
                            ps2, wkv_sb[:, ht, dt_ * P:(dt_ + 1) * P],
                            hT[:, ht, :],
                            start=(ht == 0), stop=(ht == HT - 1))
                    nc.vector.tensor_copy(
                        latT[:, dt_, c * SC:(c + 1) * SC], ps2)

        # ------- fused phase: A2 (kT/v1) + B (attention) + C (c_proj) -------
        # Wavefront over ib: the A2 chunk sc=ib produces exactly the kT/v1
        # s-range that B's query block ib needs, and C drains the OT block
        # that B(ib) finished, so PE work (A2/C) fills the ACT-bound gaps of
        # B.  All phase-B matmuls use PE tile mode (64,128): scores have a
        # 64-deep contraction and the AV matmuls are split into two 64-deep
        # halves; a uniform mode avoids per-matmul PE reconfiguration drains,
        # and the even/odd head operands live on SBUF partitions 0-63/64-127,
        # so their matmuls run concurrently on the T0/T8 array halves.
        persistB = stack.enter_context(tc.tile_pool(name="persistB", bufs=1))
        kT = persistB.tile([P, DT, S], F32R, tag="kT")
        v1 = persistB.tile([P, ST, NHL * (HD + 1)], BF16, tag="v1")
        OT = latT  # latT[ib-block] is dead once A2(ib) ran; reuse as OT

        with tc.tile_pool(name="wB", bufs=1) as wB, \
             tc.tile_pool(name="wo_p", bufs=1) as wop:
            wk_sb = wB.tile([P, CT, DL], F32R, tag="wk")
            nc.gpsimd.dma_start(wk_sb, wk.rearrange("(a p) d -> p a d", p=P))
            wv_sb = wB.tile([P, CT, DL], F32R, tag="wv")
            nc.gpsimd.dma_start(wv_sb, wv.rearrange("(a p) d -> p a d", p=P))
            wo_sb = wop.tile([P, DT, H], F32R, tag="wo")
            nc.gpsimd.dma_start(wo_sb, wo.rearrange("(a p) n -> p a n", p=P))
            # ones columns (column HD of each head's 65-wide slot); DVE
            # memset can't write bf16-from-imm reliably, stage in f32
            ones_src = wB.tile([P, ST, NHL], F32, tag="ones_src")
            nc.vector.memset(ones_src, 1.0)
            nc.vector.tensor_copy(
                v1.rearrange("p s (h e) -> p s h e", e=HD + 1)[:, :, :, HD],
                ones_src)

            def a2_chunk(sc, psB):
                # kT columns [sc*512,(sc+1)*512) and v1 s-tiles 4sc..4sc+3
                for dt_ in range(DT):
                    ps = psB.tile([P, 512], F32, tag="psk")
                    for ct in range(CT):
                        nc.tensor.matmul(
                            ps, wk_sb[:, ct, dt_ * P:(dt_ + 1) * P],
                            latT[:, ct, sc * 512:(sc + 1) * 512],
                            start=(ct == 0), stop=(ct == CT - 1))
                    nc.vector.tensor_copy(
                        kT[:, dt_, sc * 512:(sc + 1) * 512], ps)
                for s2 in range(4):
                    st = sc * 4 + s2
                    psv = psB.tile([P, 512], F32, tag="psv")
                    for ct in range(CT):
                        nc.tensor.matmul(
                            psv, latT[:, ct, st * P:(st + 1) * P],
                            wv_sb[:, ct, :],
                            start=(ct == 0), stop=(ct == CT - 1))
                    nc.vector.tensor_copy(
                        v1[:, st, :].rearrange(
                            "p (h e) -> p h e", e=HD + 1)[:, :, :HD],
                        psv.rearrange("p (h e) -> p h e", e=HD))

            def issue_scores(g, ib, jt, ps_sp):
                # both heads' scores in one 2-bank PSUM tile so a single
                # exp + mask instruction covers the pair.  Diagonal-band
                # tiles (toff > 0) are restricted to columns [toff, 512): the
                # columns below get no contribution from that j-tile, which
                # is exactly the causal mask.
                toff = (jt - 4 * ib) * P if jt >= 4 * ib else 0
                jts = slice(jt * P, (jt + 1) * P)
                iw = slice(ib * 512 + toff, (ib + 1) * 512)
                ps2 = ps_sp.tile([P, 2, 512], F32, tag="ps2")
                nc.tensor.matmul(
                    ps2[:, 0, toff:], kT[0:HD, g, jts],
                    qT[0:HD, g, iw], start=True, stop=True)
                nc.tensor.matmul(
                    ps2[:, 1, toff:], kT[HD:P, g, jts],
                    qT[HD:P, g, iw], start=True, stop=True)
                return ps2

            def attn_block(g, ib, ps_sp, ps_op, ptp, smallp,
                           first_ps2, next_block):
                se = (2 * g) * (HD + 1)
                so = (2 * g + 1) * (HD + 1)
                ibs = slice(ib * 512, (ib + 1) * 512)
                jt_max = 4 * (ib + 1)
                # One accumulator bank per (head, contraction-half): the two
                # halves run concurrently on the T0/T8 array tiles, so they
                # must not share a PSUM bank.
                po_e0 = ps_op.tile([P, 512], F32, tag="po_e0")
                po_e1 = ps_op.tile([P, 512], F32, tag="po_e1")
                po_o0 = ps_op.tile([P, 512], F32, tag="po_o0")
                po_o1 = ps_op.tile([P, 512], F32, tag="po_o1")

                ps2_next = first_ps2 if first_ps2 is not None \
                    else issue_scores(g, ib, 0, ps_sp)
                handoff = None
                for jt in range(jt_max):
                    ps2 = ps2_next
                    # issue the next scores ahead of this tile's AV so the PE
                    # never idles on the exp (nor the ACT on fresh scores);
                    # at the block end, prefetch the NEXT block's first tile
                    # so the ACT keeps flowing through the epilogue.
                    if jt + 1 < jt_max:
                        ps2_next = issue_scores(g, ib, jt + 1, ps_sp)
                    elif next_block is not None:
                        handoff = issue_scores(*next_block, 0, ps_sp)
                    toff = (jt - 4 * ib) * P if jt >= 4 * ib else 0
                    w = 512 - toff
                    pt2 = ptp.tile([P, 2, 512], BF16, tag="pt2")
                    nc.scalar.activation(
                        pt2[:, :, toff:], ps2[:, :, toff:],
                        mybir.ActivationFunctionType.Exp, scale=SCALE)
                    if jt >= 4 * ib:  # diagonal band: causal mask.  Only
                        # the 128-wide triangle band [toff, toff+128) can
                        # contain masked elements; later columns are fully
                        # below the diagonal.
                        mw = min(P, w)
                        nc.vector.tensor_mul(
                            out=pt2[:, :, toff:toff + mw],
                            in0=pt2[:, :, toff:toff + mw],
                            in1=bigmask[:, 384:384 + mw]
                            .rearrange("p (o f) -> p o f", o=1)
                            .broadcast_to((P, 2, mw)))
                    last = jt == jt_max - 1
                    nc.tensor.matmul(
                        po_e0[:HD + 1, toff:], v1[0:HD, jt, se:se + HD + 1],
                        pt2[0:HD, 0, toff:], start=(jt == 0), stop=last)
                    nc.tensor.matmul(
                        po_o0[:HD + 1, toff:], v1[0:HD, jt, so:so + HD + 1],
                        pt2[0:HD, 1, toff:], start=(jt == 0), stop=last)
                    nc.tensor.matmul(
                        po_e1[:HD + 1, toff:], v1[HD:P, jt, se:se + HD + 1],
                        pt2[HD:P, 0, toff:], start=(jt == 0), stop=last)
                    nc.tensor.matmul(
                        po_o1[:HD + 1, toff:], v1[HD:P, jt, so:so + HD + 1],
                        pt2[HD:P, 1, toff:], start=(jt == 0), stop=last)
                for po0, po1, pbase in ((po_e0, po_e1, 0),
                                        (po_o0, po_o1, HD)):
                    # DVE may read only one PSUM input per op
                    pm = smallp.tile([HD + 1, 512], F32, tag="pm")
                    nc.vector.tensor_copy(pm, po1[:HD + 1, :])
                    nc.vector.tensor_add(
                        out=pm, in0=po0[:HD + 1, :], in1=pm)
                    dn = smallp.tile([1, 512], F32, tag="dn")
                    # reciprocal_approx_fast mis-reads inputs at a nonzero
                    # base partition; stage the denominator at partition 0
                    nc.vector.tensor_copy(dn, pm[HD:HD + 1, :])
                    rc = smallp.tile([1, 512], F32, tag="rc")
                    nc.vector.reciprocal_approx_fast(out=rc, in_=dn)
                    rc64 = smallp.tile([HD, 512], F32, tag="rc64")
                    nc.gpsimd.partition_broadcast(rc64, rc)
                    nc.vector.tensor_mul(
                        out=OT[pbase:pbase + HD, g, ibs],
                        in0=pm[:HD, :], in1=rc64)
                return handoff

            def cproj_tile(m, osbp, pscp):
                osb = osbp.tile([P, H], F32, tag="osb")
                for n in range(H // 512):
                    pc = pscp.tile([P, 512], F32, tag="pc")
                    for kt in range(DT):
                        nc.tensor.matmul(
                            pc, OT[:, kt, m * P:(m + 1) * P],
                            wo_sb[:, kt, n * 512:(n + 1) * 512],
                            start=(kt == 0), stop=(kt == DT - 1))
                    # split PSUM evictions between DVE and ACT
                    if n % 2 == 0:
                        nc.vector.tensor_copy(
                            osb[:, n * 512:(n + 1) * 512], pc)
                    else:
                        nc.scalar.copy(osb[:, n * 512:(n + 1) * 512], pc)
                nc.sync.dma_start(out[m * P:(m + 1) * P, :], osb)

            with tc.tile_pool(name="psB", bufs=2, space="PSUM") as psB:
                for sc in range(4):
                    a2_chunk(sc, psB)
            with tc.tile_pool(name="ptp", bufs=3) as ptp, \
                 tc.tile_pool(name="smallp", bufs=2) as smallp, \
                 tc.tile_pool(name="ps_s", bufs=2, space="PSUM") as ps_sp, \
                 tc.tile_pool(name="ps_o", bufs=1, space="PSUM") as ps_op:
                blocks = [(g, ib) for g in range(DT) for ib in range(4)]
                carry = None
                for k, (g, ib) in enumerate(blocks):
                    nxt = blocks[k + 1] if k + 1 < len(blocks) else None
                    carry = attn_block(g, ib, ps_sp, ps_op, ptp, smallp,
                                       carry, nxt)
            with tc.tile_pool(name="osb_p", bufs=3) as osbp, \
                 tc.tile_pool(name="ps_c", bufs=2, space="PSUM") as pscp:
                for m in range(ST):
                    cproj_tile(m, osbp, pscp)

    nc.compile()
    return nc


def _get_program():
    if "nc" not in _CACHE:
        _CACHE["nc"] = build_program()
    return _CACHE["nc"]


def make_in_maps(hidden_states, wq, w_kv_down, w_k_up, w_v_up, w_out):
    """Host-side sharding: core i -> (batch i//2, head-half i%2)."""
    in_maps = []
    for i in range(8):
        b, hp = i // 2, i % 2
        sl = slice(hp * DL, (hp + 1) * DL)
        in_maps.append({
            "hs": np.ascontiguousarray(hidden_states[b]),
            "wq": np.ascontiguousarray(wq[:, sl]),
            "wkv": np.ascontiguousarray(w_kv_down),
            "wk": np.ascontiguousarray(w_k_up[:, sl]),
            "wv": np.ascontiguousarray(w_v_up[:, sl]),
            "wo": np.ascontiguousarray(w_out[sl, :]),
        })
    return in_maps


def gather(results):
    """Host-side unshard: sum the two head-half partials per batch."""
    out = np.empty((B, S, H), dtype=np.float32)
    for b in range(B):
        out[b] = results[2 * b]["out"] + results[2 * b + 1]["out"]
    return out


def kernel(hidden_states, wq, w_kv_down, w_k_up, w_v_up, w_out, _trace=False):
    nc = _get_program()
    in_maps = make_in_maps(
        np.asarray(hidden_states, dtype=np.float32),
        np.asarray(wq, dtype=np.float32),
        np.asarray(w_kv_down, dtype=np.float32),
        np.asarray(w_k_up, dtype=np.float32),
        np.asarray(w_v_up, dtype=np.float32),
        np.asarray(w_out, dtype=np.float32),
    )
    res = run_bass_kernel_spmd(nc, in_maps, list(range(8)), trace=_trace)
    out = gather(res.results)
    if _trace:
        return out, res
    return out
